# revision 70
# baseline (speedup 1.0000x reference)
"""DIEN (GRU + attention + AUGRU) Trainium2 kernel.

Data-parallel over 8 NeuronCores: each core handles a batch slice of 64.
All on-chip state is feature-major (feature dim on SBUF partitions, batch
on the free dim), so the two sequential recurrences need no per-step
transposes.

The kernel is latency-bound on the two serial recurrences, so the design
minimizes the per-step dependency chain:
  - Per-gate sigmoids read gate PSUM directly with the bias folded into
    the ACT instruction (bias ap / scale=-1 for the update gate, which is
    consumed as oz = 1-z), so no PSUM bias-add ops exist at all.
  - h-update tail is 2 vector ops: hnew = n*oz - negw, where
    negw = (oz-1)*hprev is one fused STT on the (idle) GPSIMD engine,
    issued in the tanh shadow.
  - PE issue order per step is mm_r, mm_hn, mm_z so the sigmoid and the
    n-path start as early as possible.
  - Attention (interleaved with the GRU), AUGRU input projections, the
    X^T build second half, and the masked history sum are spread across
    the step loop via a cost-budgeted op queue so no chunk-boundary PE
    burst delays the chain.
  - AUGRU attention-weight broadcast uses a block-diagonal 2-matmul
    construction per 4-step chunk (replicate rows via identity-bank
    matmul, mask to block-diagonal, ones-matmul broadcast).
  - Masked steps have weight 0 (u'=0, h unchanged), so the final AUGRU
    state IS the gathered aug_out[b, len-1] - no gather needed.
  - Matmuls are bf16 (f32 PSUM accumulation); the masked history mean
    and softmax stay f32.
"""

import sys

if "/opt/trn_rl_repo" not in sys.path:
    sys.path.insert(0, "/opt/trn_rl_repo")

from collections import deque
from contextlib import ExitStack

import ml_dtypes
import numpy as np

import concourse.bacc as bacc
import concourse.bass as bass
import concourse.mybir as mybir
import concourse.tile as tile
from concourse.bass_utils import run_bass_kernel_spmd
from concourse.masks import make_identity

f32 = mybir.dt.float32
bf16 = mybir.dt.bfloat16
i32 = mybir.dt.int32
AF = mybir.ActivationFunctionType
ALU = mybir.AluOpType
BF = ml_dtypes.bfloat16

NCORES = 8
B, T, D = 512, 200, 128
BL = B // NCORES          # 64 batch rows per core
CG = 4                    # GRU/AUGRU psum chunk: steps per chunk
NCG = T // CG             # 50
WG = CG * BL              # 256
CH = 8                    # attention chunk: steps per chunk
NCHUNK = T // CH          # 25
W = CH * BL               # 512
G3 = 3 * D


def _mm(nc, out, lhsT, rhs, start, stop, skip=False):
    nc.tensor.matmul(out, lhsT, rhs, start=start, stop=stop,
                     skip_group_check=skip)


def build_nc():
    nc = bacc.Bacc("TRN2", target_bir_lowering=False)

    hist = nc.declare_dram_parameter("hist", [BL, T, D], bf16, isOutput=False)
    item = nc.declare_dram_parameter("item", [BL, D], f32, isOutput=False)
    user = nc.declare_dram_parameter("user", [BL, D], f32, isOutput=False)
    maskd = nc.declare_dram_parameter("maskd", [BL, T], f32, isOutput=False)
    seqd = nc.declare_dram_parameter("seqd", [BL, 1], i32, isOutput=False)
    WihTd = nc.declare_dram_parameter("WihT", [D, G3], bf16, isOutput=False)
    WhhTd = nc.declare_dram_parameter("WhhT", [D, G3], bf16, isOutput=False)
    bihTd = nc.declare_dram_parameter("bihT", [D, 3], f32, isOutput=False)
    bhhTd = nc.declare_dram_parameter("bhhT", [D, 3], f32, isOutput=False)
    W0Td = nc.declare_dram_parameter("W0T", [D, 320], bf16, isOutput=False)
    b0d = nc.declare_dram_parameter("b0", [80, 1], f32, isOutput=False)
    W1Td = nc.declare_dram_parameter("W1T", [80, 40], bf16, isOutput=False)
    b1d = nc.declare_dram_parameter("b1", [40, 1], f32, isOutput=False)
    W2Td = nc.declare_dram_parameter("W2T", [40, 1], bf16, isOutput=False)
    b2d = nc.declare_dram_parameter("b2", [1, 1], f32, isOutput=False)
    augWd = nc.declare_dram_parameter("augW", [D, 6 * D], bf16, isOutput=False)
    augbd = nc.declare_dram_parameter("augb", [D, 3], f32, isOutput=False)
    outWTd = nc.declare_dram_parameter("outWT", [D, 5], bf16, isOutput=False)
    outbd = nc.declare_dram_parameter("outb", [1, 1], f32, isOutput=False)
    blkd = nc.declare_dram_parameter("blkmask", [CG, CG * BL], bf16,
                                     isOutput=False)
    outd = nc.declare_dram_parameter("out", [1, BL], f32, isOutput=True)

    with tile.TileContext(nc) as tc, ExitStack() as ctx:
        big = ctx.enter_context(tc.tile_pool(name="big", bufs=1))
        wp = ctx.enter_context(tc.tile_pool(name="wp", bufs=1))
        pp = ctx.enter_context(tc.tile_pool(name="pp", bufs=1))
        hp = ctx.enter_context(tc.tile_pool(name="hp", bufs=8))

        # XT is b-major [D, (b t)] so the hardware transpose-DMA writes it
        # contiguously; the input projections read (t, b)-ordered chunk
        # views through a strided AP.
        XT = big.tile([D, BL * T], bf16)
        gruT = big.tile([D, T * BL], bf16)
        # natural-layout history tiles kept resident so the masked history
        # sum matmuls can run interleaved with the GRU loop
        xk0 = big.tile([128, BL * D], bf16)   # t in [0,128)
        xk1 = big.tile([72, BL * D], bf16)    # t in [128,200)

        ident = pp.tile([128, 128], f32)
        make_identity(nc, ident)
        identB = pp.tile([BL, BL], bf16)
        make_identity(nc, identB)
        identB128 = pp.tile([128, 128], bf16)
        make_identity(nc, identB128)
        # 4 copies of identB side by side (AUGRU weight-broadcast trick)
        identB4 = pp.tile([BL, CG * BL], bf16)
        for j in range(CG):
            nc.scalar.copy(identB4[:, j * BL:(j + 1) * BL], identB[:, :])
        # block-diagonal mask [s, s*BL:(s+1)*BL] = 1 (host-built constant;
        # sub-partition-offset memsets are not addressable)
        blkmask = pp.tile([CG, CG * BL], bf16)
        nc.sync.dma_start(blkmask[:, :], blkd[:, :])
        ones4 = pp.tile([CG, 128], bf16)
        nc.vector.memset(ones4[:, :], 1.0)

        # ------------- weights (bf16 arrive pre-converted via DMA) -------
        WihT = wp.tile([D, G3], bf16)
        nc.sync.dma_start(WihT[:, :], WihTd[:, :])
        WhhT = wp.tile([D, G3], bf16)
        nc.sync.dma_start(WhhT[:, :], WhhTd[:, :])
        W0T = wp.tile([D, 320], bf16)
        nc.sync.dma_start(W0T[:, :], W0Td[:, :])
        W1T = wp.tile([80, 40], bf16)
        nc.sync.dma_start(W1T[:, :], W1Td[:, :])
        W2T = wp.tile([40, 1], bf16)
        nc.sync.dma_start(W2T[:, :], W2Td[:, :])
        augW = wp.tile([D, 6 * D], bf16)
        nc.sync.dma_start(augW[:, :], augWd[:, :])
        outWT = wp.tile([D, 5], bf16)
        nc.sync.dma_start(outWT[:, :], outWTd[:, :])
        WrhT, WrxT = augW[:, 0:D], augW[:, D:2 * D]
        WuhT, WuxT = augW[:, 2 * D:3 * D], augW[:, 3 * D:4 * D]
        WahT, WaxT = augW[:, 4 * D:5 * D], augW[:, 5 * D:6 * D]

        # attention layer-0 decomposition: W0·[f;q;f*q;q-f] =
        # (W0f-W0d)·f + W0p·(f*q) + (W0q+W0d)·q; the q-term is constant
        # across t per batch row and is preloaded into PSUM per chunk.
        AT = wp.tile([D, 80], bf16)
        nc.vector.tensor_sub(AT[:, :], W0T[:, 0:80], W0T[:, 240:320])
        BqT = wp.tile([D, 80], bf16)
        nc.vector.tensor_add(BqT[:, :], W0T[:, 80:160], W0T[:, 240:320])
        W0pT = W0T[:, 160:240]

        bihT = wp.tile([D, 3], f32)
        nc.sync.dma_start(bihT[:, :], bihTd[:, :])
        bhhT = wp.tile([D, 3], f32)
        nc.sync.dma_start(bhhT[:, :], bhhTd[:, :])
        brz = wp.tile([D, 2], f32)
        nc.vector.tensor_add(brz[:, :], bihT[:, 0:2], bhhT[:, 0:2])
        negbz = wp.tile([D, 1], f32)
        nc.vector.tensor_scalar_mul(negbz[:, :], brz[:, 1:2], -1.0)
        b_hn, b_in = bhhT[:, 2:3], bihT[:, 2:3]

        b0v = wp.tile([80, 1], f32)
        nc.sync.dma_start(b0v[:, :], b0d[:, :])
        b1v = wp.tile([40, 1], f32)
        nc.sync.dma_start(b1v[:, :], b1d[:, :])
        b2v = wp.tile([1, 1], f32)
        nc.sync.dma_start(b2v[:, :], b2d[:, :])
        augb = wp.tile([D, 3], f32)
        nc.sync.dma_start(augb[:, :], augbd[:, :])
        ab_r, ab_u, ab_h = augb[:, 0:1], augb[:, 1:2], augb[:, 2:3]
        outb = wp.tile([1, 1], f32)
        nc.sync.dma_start(outb[:, :], outbd[:, :])

        ones_f = wp.tile([1, 128], f32)
        nc.vector.memset(ones_f[:, :], 1.0)
        onesDB = wp.tile([D, BL], f32)
        nc.vector.memset(onesDB[:, :], 1.0)

        # ---------------- small preprocessing ---------------------------
        histT = pp.tile([D, BL], bf16)
        qT = pp.tile([D, BL], bf16)
        userT = pp.tile([D, BL], bf16)
        qRep = pp.tile([D, W], bf16)
        maskS = pp.tile([BL, T], f32)
        b2col = pp.tile([BL, 1], f32)
        histF = pp.tile([D, BL], f32)
        mst0 = pp.tile([128, BL], bf16)
        mst1 = pp.tile([72, BL], bf16)

        XTbt = XT.rearrange("p (b t) -> p b t", t=T)
        XTtb = XT.rearrange("p (b t) -> p t b", t=T)

        with tc.tile_pool(name="pre", bufs=4) as pre, \
             tc.tile_pool(name="pps", bufs=2, space="PSUM") as pps:
            nc.sync.dma_start(maskS[:, :], maskd[:, :])
            seqi = pre.tile([BL, 1], i32)
            nc.sync.dma_start(seqi[:, :], seqd[:, :])
            seqf = pre.tile([BL, 1], f32)
            nc.vector.tensor_copy(seqf[:, :], seqi[:, :])
            rsec = pre.tile([BL, 1], f32)
            nc.vector.reciprocal(rsec[:, :], seqf[:, :])
            maskSc = pre.tile([BL, T], f32)
            nc.vector.tensor_scalar_mul(maskSc[:, :], maskS[:, :], rsec[:, 0:1])

            mstp0 = pps.tile([128, BL], f32, tag="mstp")
            nc.tensor.transpose(mstp0[:, :], maskSc[:, 0:128], ident[0:BL, 0:BL])
            nc.scalar.copy(mst0[:, :], mstp0[:, :])
            mstp1 = pps.tile([128, BL], f32, tag="mstp")
            nc.tensor.transpose(mstp1[0:72, :], maskSc[:, 128:200], ident[0:BL, 0:BL])
            nc.scalar.copy(mst1[:, :], mstp1[0:72, :])

            itn = pre.tile([BL, D], f32, tag="itn")
            nc.sync.dma_start(itn[:, :], item[:, :])
            itp = pps.tile([D, BL], f32, tag="small_t")
            nc.tensor.transpose(itp[:, :], itn[:, :], ident[0:BL, 0:BL])
            nc.scalar.copy(qT[:, :], itp[:, :])
            usn = pre.tile([BL, D], f32, tag="itn")
            nc.sync.dma_start(usn[:, :], user[:, :])
            usp = pps.tile([D, BL], f32, tag="small_t")
            nc.tensor.transpose(usp[:, :], usn[:, :], ident[0:BL, 0:BL])
            nc.scalar.copy(userT[:, :], usp[:, :])
            b2p = pps.tile([BL, 1], f32, tag="small_t")
            _mm(nc, b2p[:, :], ones_f[0:1, 0:BL], b2v, start=True, stop=True)
            nc.scalar.copy(b2col[:, :], b2p[:, :])
            for s in range(CH):
                nc.scalar.copy(qRep[:, s * BL:(s + 1) * BL], qT[:, :])


            # X^T t<64: staged DMA + PE transpose upfront (all the GRU
            # needs to start). X^T t in [64,200): hardware transpose-DMAs
            # on the idle sync queue, overlapped with the GRU loop (first
            # needed at chunk 16, ~150us after they start). xk0/xk1
            # natural-layout copies feed the masked-history-sum matmuls.
            xk03 = xk0.rearrange("p (b d) -> p b d", d=D)
            xk13 = xk1.rearrange("p (b d) -> p b d", d=D)
            for b8 in range(0, BL, 8):
                nc.sync.dma_start(
                    xk03[:, b8:b8 + 8, :],
                    hist[b8:b8 + 8, 0:128, :].rearrange("b t d -> t b d"))
            for b4 in range(0, BL, 4):
                xtp = pps.tile([D, 4 * 72], bf16, tag="xtp")
                for j in range(4):
                    nc.tensor.transpose(
                        xtp[:, j * 72:(j + 1) * 72],
                        xk0[0:72, (b4 + j) * D:(b4 + j + 1) * D],
                        identB128[0:72, 0:72])
                nc.vector.tensor_copy(
                    XTbt[:, b4:b4 + 4, 0:72],
                    xtp.rearrange("p (b t) -> p b t", t=72))
            for b in range(BL):
                nc.sync.dma_start_transpose(
                    XTbt[:, b, 72:200], hist[b, 72:200, :])
            for b8 in range(0, BL, 8):
                nc.sync.dma_start(
                    xk13[:, b8:b8 + 8, :],
                    hist[b8:b8 + 8, 128:200, :].rearrange("b t d -> t b d"))

        # ---------------- GRU + interleaved deferred work ---------------
        gruT3 = gruT.rearrange("p (t b) -> p t b", b=BL)
        SIG, TANH = AF.Sigmoid, AF.Tanh
        wgt = pp.tile([BL, T], f32)

        # cost-budgeted deferred-op queues: (pe_ns, vec_ns, closure).
        # opq is high priority (attention / AUGRU prep), opq_lo fills the
        # remaining budget (X^T second half, masked history sums).
        opq = deque()
        opq_lo = deque()

        def pump(pe_budget=800.0, vec_budget=800.0):
            pe_left, vec_left = pe_budget, vec_budget
            while opq:
                pe_c, vec_c, fn = opq[0]
                if pe_c > pe_left or vec_c > vec_left:
                    break
                opq.popleft()
                fn()
                pe_left -= pe_c
                vec_left -= vec_c
            # exactly one low-priority piece per step, unconditionally:
            # X^T second-half columns MUST all be written (program order)
            # before the chunk-32 input projection reads them, and the
            # single-buffered staging slot forbids more than one per step
            if opq_lo:
                opq_lo.popleft()[2]()

        def drain():
            for q in (opq, opq_lo):
                while q:
                    q.popleft()[2]()

        with tc.tile_pool(name="gip", bufs=2, space="PSUM") as gip, \
             tc.tile_pool(name="gt", bufs=10) as gt, \
             tc.tile_pool(name="aps", bufs=1, space="PSUM") as aps, \
             tc.tile_pool(name="msc", bufs=1, space="PSUM") as msc, \
             tc.tile_pool(name="at", bufs=3) as at:
            # one bank shared by the recurrent n-gate psum, the masked
            # history sums and the attention score columns
            smalls = msc.tile([D, 512], f32, tag="smalls")
            hnt = smalls[:, 0:BL]
            histp = smalls[:, BL:3 * BL]        # [first half | second half]
            scp = smalls[0:BL, 312:512]

            # deferred masked history sums (per batch row)
            def hist_piece(b):
                def fn():
                    _mm(nc, histp[:, b:b + 1],
                        xk0[:, b * D:(b + 1) * D], mst0[:, b:b + 1],
                        start=True, stop=True)
                    _mm(nc, histp[:, BL + b:BL + b + 1],
                        xk1[:, b * D:(b + 1) * D],
                        mst1[:, b:b + 1], start=True, stop=True)
                return (400.0, 0.0, fn)

            for b in range(BL):
                opq_lo.append(hist_piece(b))

            # attention for 8-step chunk ca, split into budgeted pieces
            def attention_ops(ca):
                gc = gruT[:, ca * W:(ca + 1) * W]
                r2 = at.tile([D, W], bf16, tag="r2")
                y0p = aps.tile([80, W], f32, tag="y0")
                y0 = at.tile([80, W], bf16, tag="y0s")
                y1p = aps.tile([40, W], f32, tag="y1")
                y1 = at.tile([40, W], bf16, tag="y1s")
                ops = []
                H = W // 2
                ops.append((0.0, 360.0, lambda: nc.vector.tensor_mul(
                    r2[:, 0:H], gc[:, 0:H], qRep[:, 0:H])))
                ops.append((0.0, 360.0, lambda: nc.vector.tensor_mul(
                    r2[:, H:W], gc[:, H:W], qRep[:, H:W])))
                ops.append((640.0, 0.0, lambda: _mm(
                    nc, y0p[:, :], AT, gc, start=True, stop=False)))
                ops.append((640.0, 0.0, lambda: _mm(
                    nc, y0p[:, :], BqT, qRep, start=False, stop=False)))
                ops.append((640.0, 0.0, lambda: _mm(
                    nc, y0p[:, :], W0pT, r2, start=False, stop=True)))
                ops.append((0.0, 440.0, lambda: nc.vector.tensor_scalar(
                    y0[:, 0:H], y0p[:, 0:H], b0v[:, 0:1], 0.0,
                    ALU.add, ALU.max)))
                ops.append((0.0, 440.0, lambda: nc.vector.tensor_scalar(
                    y0[:, H:W], y0p[:, H:W], b0v[:, 0:1], 0.0,
                    ALU.add, ALU.max)))

                def f_y1():
                    _mm(nc, y1p[:, :], W1T, y0, start=True, stop=True)
                ops.append((320.0, 0.0, f_y1))
                ops.append((0.0, 560.0, lambda: nc.vector.tensor_scalar(
                    y1[:, :], y1p[:, :], b1v[:, 0:1], 0.0,
                    ALU.add, ALU.max)))

                def f_scp(s0):
                    def fn():
                        for s in (s0, s0 + 1):
                            t = ca * CH + s
                            _mm(nc, scp[:, t:t + 1],
                                y1[:, s * BL:(s + 1) * BL],
                                W2T, start=True, stop=True)
                    return fn
                for s0 in range(0, CH, 2):
                    ops.append((420.0, 0.0, f_scp(s0)))
                return ops

            def iproj(c):
                XTc = XTtb[:, c * CG:(c + 1) * CG, :]
                prz = gip.tile([D, 2 * WG], f32, tag="girz")
                pn = gip.tile([D, WG], f32, tag="gin")
                _mm(nc, prz[:, 0:WG], WihT[:, 0:D], XTc,
                    start=True, stop=True)
                _mm(nc, prz[:, WG:2 * WG], WihT[:, D:2 * D], XTc,
                    start=True, stop=True)
                _mm(nc, pn[:, :], WihT[:, 2 * D:G3], XTc,
                    start=True, stop=True)
                return prz, pn

            cur = iproj(0)
            nxt = [None]
            for c in range(NCG):
                prz, pn = cur
                pn3 = pn.rearrange("p (s g) -> p s g", g=BL)
                for s in range(CG):
                    t = c * CG + s
                    r = gt.tile([D, BL], f32, tag="r")
                    oz = gt.tile([D, BL], f32, tag="oz")
                    if t > 0:
                        hprev = gruT3[:, t - 1, :]
                        # PSUM reads wait for every PE write to the same
                        # bank issued before them, so each sigmoid is
                        # issued immediately after its own gate matmul.
                        _mm(nc, prz[:, s * BL:(s + 1) * BL], WhhT[:, 0:D],
                            hprev, start=False, stop=True, skip=True)
                        nc.scalar.activation(r[:, :],
                                             prz[:, s * BL:(s + 1) * BL],
                                             SIG, bias=brz[:, 0:1])
                        hn = hnt
                        _mm(nc, hn[:, :], WhhT[:, 2 * D:G3], hprev,
                            start=True, stop=True)
                        tmp = gt.tile([D, BL], f32, tag="tmp")
                        nc.vector.scalar_tensor_tensor(
                            tmp[:, :], hn[:, :], b_hn, r[:, :],
                            ALU.add, ALU.mult)
                        _mm(nc, prz[:, WG + s * BL:WG + (s + 1) * BL],
                            WhhT[:, D:2 * D], hprev,
                            start=False, stop=True, skip=True)
                        nc.scalar.activation(
                            oz[:, :], prz[:, WG + s * BL:WG + (s + 1) * BL],
                            SIG, bias=negbz[:, 0:1], scale=-1.0)
                        nc.vector.tensor_add(pn3[:, s, :], tmp[:, :],
                                             pn3[:, s, :])
                        ozm1 = gt.tile([D, BL], f32, tag="ozm1")
                        nc.vector.tensor_scalar_add(ozm1[:, :], oz[:, :],
                                                    -1.0)
                        negw = gt.tile([D, BL], f32, tag="negw")
                        nc.gpsimd.tensor_mul(negw[:, :], ozm1[:, :], hprev)
                    else:
                        nc.scalar.activation(r[:, :],
                                             prz[:, s * BL:(s + 1) * BL],
                                             SIG, bias=brz[:, 0:1])
                        nc.scalar.activation(
                            oz[:, :], prz[:, WG + s * BL:WG + (s + 1) * BL],
                            SIG, bias=negbz[:, 0:1], scale=-1.0)
                        nc.vector.scalar_tensor_tensor(
                            pn3[:, s, :], r[:, :], b_hn, pn3[:, s, :],
                            ALU.mult, ALU.add)
                    n = gt.tile([D, BL], f32, tag="n")
                    nc.scalar.activation(n[:, :], pn3[:, s, :], TANH,
                                         bias=b_in)
                    if t > 0:
                        nm = gt.tile([D, BL], f32, tag="nm")
                        nc.vector.tensor_mul(nm[:, :], n[:, :], oz[:, :])
                        nc.vector.tensor_sub(gruT3[:, t, :], nm[:, :],
                                             negw[:, :])
                    else:
                        nc.vector.tensor_mul(gruT3[:, t, :], n[:, :],
                                             oz[:, :])
                    if s == 1 and c + 1 < NCG:
                        nxt[0] = iproj(c + 1)
                    pump()
                cur = nxt[0]
                if c % 2 == 1:
                    for op in attention_ops(c // 2):
                        opq.append(op)

            drain()

            # finalize masked history mean
            nc.scalar.copy(histF[:, :], histp[:, 0:BL])
            nc.vector.tensor_add(histF[:, :], histF[:, :], histp[:, BL:2 * BL])
            nc.scalar.copy(histT[:, :], histF[:, :])

            # masked softmax over t (b-major); last MLP layer ReLU'd w/ b2
            rawr = at.tile([BL, T], f32, tag="rawr")
            nc.scalar.activation(rawr[:, :], scp[:, :], AF.Relu,
                                 bias=b2col[:, 0:1])
            rawm = at.tile([BL, T], f32, tag="rawm")
            nc.vector.tensor_mul(rawm[:, :], rawr[:, :], maskS[:, :])
            mxn = at.tile([BL, 1], f32, tag="mxn")
            nc.vector.tensor_reduce(mxn[:, :], rawm[:, :],
                                    axis=mybir.AxisListType.X,
                                    op=ALU.max, negate=True)
            ex = at.tile([BL, T], f32, tag="ex")
            nc.scalar.activation(ex[:, :], rawr[:, :], AF.Exp, bias=mxn[:, 0:1])
            em = at.tile([BL, T], f32, tag="em")
            nc.vector.tensor_mul(em[:, :], ex[:, :], maskS[:, :])
            sm = at.tile([BL, 1], f32, tag="sm")
            nc.vector.tensor_reduce(sm[:, :], em[:, :],
                                    axis=mybir.AxisListType.X, op=ALU.add)
            rs = at.tile([BL, 1], f32, tag="rs")
            nc.vector.reciprocal(rs[:, :], sm[:, :])
            nc.vector.tensor_scalar_mul(wgt[:, :], em[:, :], rs[:, 0:1])

        wgt_bf = pp.tile([BL, T], bf16)
        nc.scalar.copy(wgt_bf[:, :], wgt[:, :])

        # ---------------- AUGRU ----------------------------------------
        hA = [None]
        with tc.tile_pool(name="axp", bufs=2, space="PSUM") as axp, \
             tc.tile_pool(name="abp", bufs=2, space="PSUM") as abp, \
             tc.tile_pool(name="ut", bufs=10) as ut:

            def aug_prep(c):
                gc = gruT[:, c * WG:(c + 1) * WG]
                pru = axp.tile([D, 2 * WG], f32, tag="pxru")
                pxh = axp.tile([D, WG], f32, tag="pxh")
                repp = abp.tile([CG, CG * BL], f32, tag="repp")
                masked = ut.tile([CG, CG * BL], bf16, tag="maskedw")
                pab = abp.tile([D, WG], f32, tag="pab")
                ops = []
                ops.append((420.0, 0.0, lambda: _mm(
                    nc, pru[:, 0:WG], WrxT, gc, start=True, stop=True)))
                ops.append((420.0, 0.0, lambda: _mm(
                    nc, pru[:, WG:2 * WG], WuxT, gc, start=True, stop=True)))
                ops.append((420.0, 0.0, lambda: _mm(
                    nc, pxh[:, :], WaxT, gc, start=True, stop=True)))

                def f_rep():
                    _mm(nc, repp[:, :], wgt_bf[:, c * CG:(c + 1) * CG],
                        identB4, start=True, stop=True)
                    nc.vector.tensor_mul(masked[:, :], repp[:, :],
                                         blkmask[:, :])
                ops.append((230.0, 450.0, f_rep))
                ops.append((330.0, 0.0, lambda: _mm(
                    nc, pab[:, :], ones4, masked, start=True, stop=True)))
                return (pru, pxh, pab), ops

            opq.clear()
            cur, ops0 = aug_prep(0)
            for _, _, fn in ops0:
                fn()
            nxt = [None]
            for c in range(NCG):
                pru, pxh, pab = cur
                pxh3 = pxh.rearrange("p (s g) -> p s g", g=BL)
                for s in range(CG):
                    t = c * CG + s
                    hAg = hA[0]
                    r = ut.tile([D, BL], f32, tag="ar")
                    u = ut.tile([D, BL], f32, tag="au")
                    if t > 0:
                        _mm(nc, pru[:, s * BL:(s + 1) * BL], WrhT, hAg,
                            start=False, stop=True, skip=True)
                        nc.scalar.activation(r[:, :],
                                             pru[:, s * BL:(s + 1) * BL],
                                             SIG, bias=ab_r)
                        _mm(nc, pru[:, WG + s * BL:WG + (s + 1) * BL],
                            WuhT, hAg, start=False, stop=True, skip=True)
                    else:
                        nc.scalar.activation(r[:, :],
                                             pru[:, s * BL:(s + 1) * BL],
                                             SIG, bias=ab_r)
                    nc.scalar.activation(u[:, :],
                                         pru[:, WG + s * BL:WG + (s + 1) * BL],
                                         SIG, bias=ab_u)
                    if t > 0:
                        hr = ut.tile([D, BL], bf16, tag="ahr")
                        nc.vector.tensor_mul(hr[:, :], hAg, r[:, :])
                        _mm(nc, pxh3[:, s, :], WahT, hr,
                            start=False, stop=True, skip=True)
                    up = ut.tile([D, BL], f32, tag="aup")
                    nc.vector.tensor_mul(up[:, :],
                                         pab[:, s * BL:(s + 1) * BL], u[:, :])
                    hh = ut.tile([D, BL], f32, tag="ahh")
                    nc.scalar.activation(hh[:, :], pxh3[:, s, :], TANH,
                                         bias=ab_h)
                    hnew = hp.tile([D, BL], bf16, tag="hA")
                    if t > 0:
                        dd = ut.tile([D, BL], f32, tag="add")
                        nc.vector.tensor_sub(dd[:, :], hh[:, :], hAg)
                        ud = ut.tile([D, BL], f32, tag="aud")
                        nc.vector.tensor_mul(ud[:, :], up[:, :], dd[:, :])
                        nc.vector.tensor_add(hnew[:, :], hAg, ud[:, :])
                    else:
                        nc.vector.tensor_mul(hnew[:, :], up[:, :], hh[:, :])
                    hA[0] = hnew
                    if s == 0 and c + 1 < NCG:
                        prep, ops = aug_prep(c + 1)
                        nxt[0] = prep
                        for op in ops:
                            opq.append(op)
                    pump(pe_budget=900.0, vec_budget=800.0)
                while opq:
                    _, _, fn = opq.popleft()
                    fn()
                cur = nxt[0]

        # ---------------- output layer ---------------------------------
        with tc.tile_pool(name="ops", bufs=1, space="PSUM") as ops_p, \
             tc.tile_pool(name="ot", bufs=1) as ot:
            ih = ot.tile([D, BL], bf16)
            nc.vector.tensor_mul(ih[:, :], qT[:, :], histF[:, :])
            po = ops_p.tile([1, BL], f32)
            pieces = [userT, qT, histT, ih, hA[0]]
            for g, piece in enumerate(pieces):
                _mm(nc, po[:, :], outWT[:, g:g + 1], piece,
                    start=(g == 0), stop=(g == 4))
            outs = ot.tile([1, BL], f32)
            nc.scalar.activation(outs[:, :], po[:, :], AF.Identity,
                                 bias=outb[:, 0:1])
            nc.sync.dma_start(outd[:, :], outs[:, :])

    nc.finalize()
    return nc


_NC = None


def _get_nc():
    global _NC
    if _NC is None:
        _NC = build_nc()
    return _NC


def make_in_maps(inputs):
    """Slice full inputs into per-core input maps (host-side layout only)."""
    f = {k: np.asarray(v) for k, v in inputs.items()}
    WihT = np.ascontiguousarray(f["gru_Wih"].T)          # (128, 384)
    WhhT = np.ascontiguousarray(f["gru_Whh"].T)
    bihT = np.ascontiguousarray(f["gru_bih"].reshape(3, D).T)  # (128, 3)
    bhhT = np.ascontiguousarray(f["gru_bhh"].reshape(3, D).T)
    W0T = np.ascontiguousarray(
        f["attn_W0"].T.reshape(4, D, 80).transpose(1, 0, 2).reshape(D, 320))
    b0 = np.ascontiguousarray(f["attn_b0"].reshape(80, 1))
    W1T = np.ascontiguousarray(f["attn_W1"].T)           # (80, 40)
    b1 = np.ascontiguousarray(f["attn_b1"].reshape(40, 1))
    W2T = np.ascontiguousarray(f["attn_W2"].T)           # (40, 1)
    b2 = f["attn_b2"].reshape(1, 1)
    augW = np.concatenate(
        [np.ascontiguousarray(f[k][:, p * D:(p + 1) * D].T)
         for k in ("aug_Wr", "aug_Wu", "aug_Wh") for p in (0, 1)],
        axis=1)                                          # (128, 768)
    augb = np.stack([f["aug_br"], f["aug_bu"], f["aug_bh"]], axis=1)  # (128,3)
    outWT = np.ascontiguousarray(f["out_W"].reshape(5, D).T)          # (128,5)
    outb = f["out_b"].reshape(1, 1)

    shared_bf = dict(WihT=WihT, WhhT=WhhT, W0T=W0T, W1T=W1T, W2T=W2T,
                     augW=augW, outWT=outWT)
    shared = dict(bihT=bihT, bhhT=bhhT, b0=b0, b1=b1, b2=b2, augb=augb,
                  outb=outb)
    shared = {k: np.ascontiguousarray(v.astype(np.float32)) for k, v in
              shared.items()}
    shared.update({k: np.ascontiguousarray(v.astype(BF)) for k, v in
                   shared_bf.items()})
    blk = np.zeros((CG, CG * BL), dtype=BF)
    for s in range(CG):
        blk[s, s * BL:(s + 1) * BL] = 1
    shared["blkmask"] = blk

    in_maps = []
    for c in range(NCORES):
        s = slice(c * BL, (c + 1) * BL)
        m = dict(shared)
        m["hist"] = np.ascontiguousarray(
            f["item_historical_embedding"][s].astype(BF))
        m["item"] = np.ascontiguousarray(f["item_embedding"][s].astype(np.float32))
        m["user"] = np.ascontiguousarray(f["user_embedding"][s].astype(np.float32))
        m["maskd"] = np.ascontiguousarray(f["mask"][s].astype(np.float32))
        m["seqd"] = np.ascontiguousarray(
            f["sequential_length"][s].reshape(BL, 1).astype(np.int32))
        in_maps.append(m)
    return in_maps


def kernel(**inputs) -> np.ndarray:
    nc = _get_nc()
    in_maps = make_in_maps(inputs)
    res = run_bass_kernel_spmd(nc, in_maps, list(range(NCORES)))
    return np.concatenate(
        [np.asarray(res.results[c]["out"]).reshape(BL) for c in range(NCORES)])


# revision 75
# speedup vs baseline: 1.0671x; 1.0671x over previous
"""DIEN (GRU + attention + AUGRU) Trainium2 kernel.

Data-parallel over 8 NeuronCores: each core handles a batch slice of 64.
All on-chip state is feature-major (feature dim on SBUF partitions, batch
on the free dim), so the two sequential recurrences need no per-step
transposes.

The kernel is latency-bound on the two serial recurrences, so the design
minimizes the per-step dependency chain:
  - Per-gate sigmoids read gate PSUM directly with the bias folded into
    the ACT instruction (bias ap / scale=-1 for the update gate, which is
    consumed as oz = 1-z), so no PSUM bias-add ops exist at all.
  - h-update tail is 2 vector ops: hnew = n*oz - negw, where
    negw = (oz-1)*hprev is one fused STT on the (idle) GPSIMD engine,
    issued in the tanh shadow.
  - PE issue order per step is mm_r, mm_hn, mm_z so the sigmoid and the
    n-path start as early as possible.
  - Attention (interleaved with the GRU), AUGRU input projections, the
    X^T build second half, and the masked history sum are spread across
    the step loop via a cost-budgeted op queue so no chunk-boundary PE
    burst delays the chain.
  - AUGRU attention-weight broadcast uses a block-diagonal 2-matmul
    construction per 4-step chunk (replicate rows via identity-bank
    matmul, mask to block-diagonal, ones-matmul broadcast).
  - Masked steps have weight 0 (u'=0, h unchanged), so the final AUGRU
    state IS the gathered aug_out[b, len-1] - no gather needed.
  - Matmuls are bf16 (f32 PSUM accumulation); the masked history mean
    and softmax stay f32.
"""

import sys

if "/opt/trn_rl_repo" not in sys.path:
    sys.path.insert(0, "/opt/trn_rl_repo")

from collections import deque
from contextlib import ExitStack

import ml_dtypes
import numpy as np

import concourse.bacc as bacc
import concourse.bass as bass
import concourse.mybir as mybir
import concourse.tile as tile
from concourse.bass_utils import run_bass_kernel_spmd
from concourse.masks import make_identity

f32 = mybir.dt.float32
bf16 = mybir.dt.bfloat16
i32 = mybir.dt.int32
AF = mybir.ActivationFunctionType
ALU = mybir.AluOpType
BF = ml_dtypes.bfloat16

NCORES = 8
B, T, D = 512, 200, 128
BL = B // NCORES          # 64 batch rows per core
CG = 4                    # GRU/AUGRU psum chunk: steps per chunk
NCG = T // CG             # 50
WG = CG * BL              # 256
CH = 8                    # attention chunk: steps per chunk
NCHUNK = T // CH          # 25
W = CH * BL               # 512
G3 = 3 * D


def _mm(nc, out, lhsT, rhs, start, stop, skip=False):
    nc.tensor.matmul(out, lhsT, rhs, start=start, stop=stop,
                     skip_group_check=skip)


def build_nc():
    nc = bacc.Bacc("TRN2", target_bir_lowering=False)

    hist = nc.declare_dram_parameter("hist", [BL, T, D], bf16, isOutput=False)
    item = nc.declare_dram_parameter("item", [BL, D], f32, isOutput=False)
    user = nc.declare_dram_parameter("user", [BL, D], f32, isOutput=False)
    maskd = nc.declare_dram_parameter("maskd", [BL, T], f32, isOutput=False)
    seqd = nc.declare_dram_parameter("seqd", [BL, 1], i32, isOutput=False)
    WihTd = nc.declare_dram_parameter("WihT", [D, G3], bf16, isOutput=False)
    WhhTd = nc.declare_dram_parameter("WhhT", [D, G3], bf16, isOutput=False)
    bihTd = nc.declare_dram_parameter("bihT", [D, 3], f32, isOutput=False)
    bhhTd = nc.declare_dram_parameter("bhhT", [D, 3], f32, isOutput=False)
    W0Td = nc.declare_dram_parameter("W0T", [D, 320], bf16, isOutput=False)
    b0d = nc.declare_dram_parameter("b0", [80, 1], f32, isOutput=False)
    W1Td = nc.declare_dram_parameter("W1T", [80, 40], bf16, isOutput=False)
    b1d = nc.declare_dram_parameter("b1", [40, 1], f32, isOutput=False)
    W2Td = nc.declare_dram_parameter("W2T", [40, 1], bf16, isOutput=False)
    b2d = nc.declare_dram_parameter("b2", [1, 1], f32, isOutput=False)
    augWd = nc.declare_dram_parameter("augW", [D, 6 * D], bf16, isOutput=False)
    augbd = nc.declare_dram_parameter("augb", [D, 3], f32, isOutput=False)
    outWTd = nc.declare_dram_parameter("outWT", [D, 5], bf16, isOutput=False)
    outbd = nc.declare_dram_parameter("outb", [1, 1], f32, isOutput=False)
    blkd = nc.declare_dram_parameter("blkmask", [CG, CG * BL], bf16,
                                     isOutput=False)
    outd = nc.declare_dram_parameter("out", [1, BL], f32, isOutput=True)

    with tile.TileContext(nc) as tc, ExitStack() as ctx:
        big = ctx.enter_context(tc.tile_pool(name="big", bufs=1))
        wp = ctx.enter_context(tc.tile_pool(name="wp", bufs=1))
        pp = ctx.enter_context(tc.tile_pool(name="pp", bufs=1))
        hp = ctx.enter_context(tc.tile_pool(name="hp", bufs=8))

        # XT is b-major [D, (b t)] so the hardware transpose-DMA writes it
        # contiguously; the input projections read (t, b)-ordered chunk
        # views through a strided AP.
        XT = big.tile([D, BL * T], bf16)
        gruT = big.tile([D, T * BL], bf16)
        # natural-layout history tiles kept resident so the masked history
        # sum matmuls can run interleaved with the GRU loop
        xk0 = big.tile([128, BL * D], bf16)   # t in [0,128)
        xk1 = big.tile([72, BL * D], bf16)    # t in [128,200)

        ident = pp.tile([128, 128], f32)
        make_identity(nc, ident)
        identB = pp.tile([BL, BL], bf16)
        make_identity(nc, identB)
        identB128 = pp.tile([128, 128], bf16)
        make_identity(nc, identB128)
        # 4 copies of identB side by side (AUGRU weight-broadcast trick)
        identB4 = pp.tile([BL, CG * BL], bf16)
        for j in range(CG):
            nc.scalar.copy(identB4[:, j * BL:(j + 1) * BL], identB[:, :])
        # block-diagonal mask [s, s*BL:(s+1)*BL] = 1 (host-built constant;
        # sub-partition-offset memsets are not addressable)
        blkmask = pp.tile([CG, CG * BL], bf16)
        nc.sync.dma_start(blkmask[:, :], blkd[:, :])
        ones4 = pp.tile([CG, 128], bf16)
        nc.vector.memset(ones4[:, :], 1.0)

        # ------------- weights (bf16 arrive pre-converted via DMA) -------
        WihT = wp.tile([D, G3], bf16)
        nc.sync.dma_start(WihT[:, :], WihTd[:, :])
        WhhT = wp.tile([D, G3], bf16)
        nc.sync.dma_start(WhhT[:, :], WhhTd[:, :])
        W0T = wp.tile([D, 320], bf16)
        nc.sync.dma_start(W0T[:, :], W0Td[:, :])
        W1T = wp.tile([80, 40], bf16)
        nc.sync.dma_start(W1T[:, :], W1Td[:, :])
        W2T = wp.tile([40, 1], bf16)
        nc.sync.dma_start(W2T[:, :], W2Td[:, :])
        augW = wp.tile([D, 6 * D], bf16)
        nc.sync.dma_start(augW[:, :], augWd[:, :])
        outWT = wp.tile([D, 5], bf16)
        nc.sync.dma_start(outWT[:, :], outWTd[:, :])
        WrhT, WrxT = augW[:, 0:D], augW[:, D:2 * D]
        WuhT, WuxT = augW[:, 2 * D:3 * D], augW[:, 3 * D:4 * D]
        WahT, WaxT = augW[:, 4 * D:5 * D], augW[:, 5 * D:6 * D]

        # attention layer-0 decomposition: W0·[f;q;f*q;q-f] =
        # (W0f-W0d)·f + W0p·(f*q) + (W0q+W0d)·q; the q-term is constant
        # across t per batch row and is preloaded into PSUM per chunk.
        AT = wp.tile([D, 80], bf16)
        nc.vector.tensor_sub(AT[:, :], W0T[:, 0:80], W0T[:, 240:320])
        BqT = wp.tile([D, 80], bf16)
        nc.vector.tensor_add(BqT[:, :], W0T[:, 80:160], W0T[:, 240:320])
        W0pT = W0T[:, 160:240]

        bihT = wp.tile([D, 3], f32)
        nc.sync.dma_start(bihT[:, :], bihTd[:, :])
        bhhT = wp.tile([D, 3], f32)
        nc.sync.dma_start(bhhT[:, :], bhhTd[:, :])
        brz = wp.tile([D, 2], f32)
        nc.vector.tensor_add(brz[:, :], bihT[:, 0:2], bhhT[:, 0:2])
        negbz = wp.tile([D, 1], f32)
        nc.vector.tensor_scalar_mul(negbz[:, :], brz[:, 1:2], -1.0)
        b_hn, b_in = bhhT[:, 2:3], bihT[:, 2:3]

        b0v = wp.tile([80, 1], f32)
        nc.sync.dma_start(b0v[:, :], b0d[:, :])
        b1v = wp.tile([40, 1], f32)
        nc.sync.dma_start(b1v[:, :], b1d[:, :])
        b2v = wp.tile([1, 1], f32)
        nc.sync.dma_start(b2v[:, :], b2d[:, :])
        augb = wp.tile([D, 3], f32)
        nc.sync.dma_start(augb[:, :], augbd[:, :])
        ab_r, ab_u, ab_h = augb[:, 0:1], augb[:, 1:2], augb[:, 2:3]
        outb = wp.tile([1, 1], f32)
        nc.sync.dma_start(outb[:, :], outbd[:, :])

        ones_f = wp.tile([1, 128], f32)
        nc.vector.memset(ones_f[:, :], 1.0)
        onesDB = wp.tile([D, BL], f32)
        nc.vector.memset(onesDB[:, :], 1.0)

        # ---------------- small preprocessing ---------------------------
        histT = pp.tile([D, BL], bf16)
        qT = pp.tile([D, BL], bf16)
        userT = pp.tile([D, BL], bf16)
        qRep = pp.tile([D, W], bf16)
        maskS = pp.tile([BL, T], f32)
        b2col = pp.tile([BL, 1], f32)
        histF = pp.tile([D, BL], f32)
        mst0 = pp.tile([128, BL], bf16)
        mst1 = pp.tile([72, BL], bf16)

        XTbt = XT.rearrange("p (b t) -> p b t", t=T)
        XTtb = XT.rearrange("p (b t) -> p t b", t=T)

        with tc.tile_pool(name="pre", bufs=4) as pre, \
             tc.tile_pool(name="pps", bufs=2, space="PSUM") as pps:
            nc.sync.dma_start(maskS[:, :], maskd[:, :])
            seqi = pre.tile([BL, 1], i32)
            nc.sync.dma_start(seqi[:, :], seqd[:, :])
            seqf = pre.tile([BL, 1], f32)
            nc.vector.tensor_copy(seqf[:, :], seqi[:, :])
            rsec = pre.tile([BL, 1], f32)
            nc.vector.reciprocal(rsec[:, :], seqf[:, :])
            maskSc = pre.tile([BL, T], f32)
            nc.vector.tensor_scalar_mul(maskSc[:, :], maskS[:, :], rsec[:, 0:1])

            mstp0 = pps.tile([128, BL], f32, tag="mstp")
            nc.tensor.transpose(mstp0[:, :], maskSc[:, 0:128], ident[0:BL, 0:BL])
            nc.scalar.copy(mst0[:, :], mstp0[:, :])
            mstp1 = pps.tile([128, BL], f32, tag="mstp")
            nc.tensor.transpose(mstp1[0:72, :], maskSc[:, 128:200], ident[0:BL, 0:BL])
            nc.scalar.copy(mst1[:, :], mstp1[0:72, :])

            itn = pre.tile([BL, D], f32, tag="itn")
            nc.sync.dma_start(itn[:, :], item[:, :])
            itp = pps.tile([D, BL], f32, tag="small_t")
            nc.tensor.transpose(itp[:, :], itn[:, :], ident[0:BL, 0:BL])
            nc.scalar.copy(qT[:, :], itp[:, :])
            usn = pre.tile([BL, D], f32, tag="itn")
            nc.sync.dma_start(usn[:, :], user[:, :])
            usp = pps.tile([D, BL], f32, tag="small_t")
            nc.tensor.transpose(usp[:, :], usn[:, :], ident[0:BL, 0:BL])
            nc.scalar.copy(userT[:, :], usp[:, :])
            b2p = pps.tile([BL, 1], f32, tag="small_t")
            _mm(nc, b2p[:, :], ones_f[0:1, 0:BL], b2v, start=True, stop=True)
            nc.scalar.copy(b2col[:, :], b2p[:, :])
            for s in range(CH):
                nc.scalar.copy(qRep[:, s * BL:(s + 1) * BL], qT[:, :])


            # X^T t<64: staged DMA + PE transpose upfront (all the GRU
            # needs to start). X^T t in [64,200): hardware transpose-DMAs
            # on the idle sync queue, overlapped with the GRU loop (first
            # needed at chunk 16, ~150us after they start). xk0/xk1
            # natural-layout copies feed the masked-history-sum matmuls.
            xk03 = xk0.rearrange("p (b d) -> p b d", d=D)
            xk13 = xk1.rearrange("p (b d) -> p b d", d=D)
            for b8 in range(0, BL, 8):
                nc.sync.dma_start(
                    xk03[:, b8:b8 + 8, :],
                    hist[b8:b8 + 8, 0:128, :].rearrange("b t d -> t b d"))
            for b4 in range(0, BL, 4):
                xtp = pps.tile([D, 4 * 64], bf16, tag="xtp")
                for j in range(4):
                    nc.tensor.transpose(
                        xtp[:, j * 64:(j + 1) * 64],
                        xk0[0:64, (b4 + j) * D:(b4 + j + 1) * D],
                        identB128[0:64, 0:64])
                nc.vector.tensor_copy(
                    XTbt[:, b4:b4 + 4, 0:64],
                    xtp.rearrange("p (b t) -> p b t", t=64))
            for b8 in range(0, BL, 8):
                nc.sync.dma_start(
                    xk13[:, b8:b8 + 8, :],
                    hist[b8:b8 + 8, 128:200, :].rearrange("b t d -> t b d"))

        # ---------------- GRU + interleaved deferred work ---------------
        gruT3 = gruT.rearrange("p (t b) -> p t b", b=BL)
        SIG, TANH = AF.Sigmoid, AF.Tanh
        wgt = pp.tile([BL, T], f32)

        # cost-budgeted deferred-op queues: (pe_ns, vec_ns, closure).
        # opq is high priority (attention / AUGRU prep), opq_lo fills the
        # remaining budget (X^T second half, masked history sums).
        opq = deque()
        opq_lo = deque()

        def pump(pe_budget=800.0, vec_budget=800.0):
            pe_left, vec_left = pe_budget, vec_budget
            while opq:
                pe_c, vec_c, fn = opq[0]
                if pe_c > pe_left or vec_c > vec_left:
                    break
                opq.popleft()
                fn()
                pe_left -= pe_c
                vec_left -= vec_c
            # exactly one low-priority piece per step, unconditionally:
            # X^T second-half columns MUST all be written (program order)
            # before the chunk-32 input projection reads them, and the
            # single-buffered staging slot forbids more than one per step
            if opq_lo:
                opq_lo.popleft()[2]()

        def drain():
            for q in (opq, opq_lo):
                while q:
                    q.popleft()[2]()

        with tc.tile_pool(name="gip", bufs=2, space="PSUM") as gip, \
             tc.tile_pool(name="gt", bufs=10) as gt, \
             tc.tile_pool(name="aps", bufs=1, space="PSUM") as aps, \
             tc.tile_pool(name="msc", bufs=1, space="PSUM") as msc, \
             tc.tile_pool(name="xtb", bufs=1, space="PSUM") as xtb, \
             tc.tile_pool(name="at", bufs=3) as at:
            # one bank shared by the recurrent n-gate psum, the masked
            # history sums and the attention score columns
            smalls = msc.tile([D, 512], f32, tag="smalls")
            hnt = smalls[:, 0:BL]
            histp = smalls[:, BL:3 * BL]        # [first half | second half]
            scp = smalls[0:BL, 312:512]
            xtp2s = xtb.tile([D, 256], bf16, tag="xtp2")

            # deferred X^T build pieces (pairs of batch rows per piece,
            # staged through one psum bank) + masked history sums.
            # Deadlines (guaranteed by exactly-1-piece-per-step pumping):
            # t in [64,128) pieces done by step 32 (needed at chunk 16),
            # t in [128,200) by step 64 (needed at chunk 32).
            def xt_pair(src, p0, t0, tn, b):
                def fn():
                    for j in (0, 1):
                        nc.tensor.transpose(
                            xtp2s[:, j * 128:j * 128 + tn],
                            src[p0:p0 + tn, (b + j) * D:(b + j + 1) * D],
                            identB128[p0:p0 + tn, p0:p0 + tn])
                    nc.vector.tensor_copy(
                        XTbt[:, b:b + 2, t0:t0 + tn],
                        xtp2s.rearrange("p (b t) -> p b t", b=2)[:, :, 0:tn])
                return (470.0, 300.0, fn)

            def hist_piece(b):
                def fn():
                    _mm(nc, histp[:, b:b + 1],
                        xk0[:, b * D:(b + 1) * D], mst0[:, b:b + 1],
                        start=True, stop=True)
                    _mm(nc, histp[:, BL + b:BL + b + 1],
                        xk1[:, b * D:(b + 1) * D],
                        mst1[:, b:b + 1], start=True, stop=True)
                return (400.0, 0.0, fn)

            for b in range(0, BL, 2):
                opq_lo.append(xt_pair(xk0, 64, 64, 64, b))
            for b in range(0, BL, 2):
                opq_lo.append(xt_pair(xk1, 0, 128, 72, b))
            for b in range(BL):
                opq_lo.append(hist_piece(b))

            # attention for 8-step chunk ca, split into budgeted pieces
            def attention_ops(ca):
                gc = gruT[:, ca * W:(ca + 1) * W]
                r2 = at.tile([D, W], bf16, tag="r2")
                y0p = aps.tile([80, W], f32, tag="y0")
                y0 = at.tile([80, W], bf16, tag="y0s")
                y1p = aps.tile([40, W], f32, tag="y1")
                y1 = at.tile([40, W], bf16, tag="y1s")
                ops = []
                H = W // 2
                ops.append((0.0, 360.0, lambda: nc.vector.tensor_mul(
                    r2[:, 0:H], gc[:, 0:H], qRep[:, 0:H])))
                ops.append((0.0, 360.0, lambda: nc.vector.tensor_mul(
                    r2[:, H:W], gc[:, H:W], qRep[:, H:W])))
                ops.append((640.0, 0.0, lambda: _mm(
                    nc, y0p[:, :], AT, gc, start=True, stop=False)))
                ops.append((640.0, 0.0, lambda: _mm(
                    nc, y0p[:, :], BqT, qRep, start=False, stop=False)))
                ops.append((640.0, 0.0, lambda: _mm(
                    nc, y0p[:, :], W0pT, r2, start=False, stop=True)))
                ops.append((0.0, 440.0, lambda: nc.vector.tensor_scalar(
                    y0[:, 0:H], y0p[:, 0:H], b0v[:, 0:1], 0.0,
                    ALU.add, ALU.max)))
                ops.append((0.0, 440.0, lambda: nc.vector.tensor_scalar(
                    y0[:, H:W], y0p[:, H:W], b0v[:, 0:1], 0.0,
                    ALU.add, ALU.max)))

                def f_y1():
                    _mm(nc, y1p[:, :], W1T, y0, start=True, stop=True)
                ops.append((320.0, 0.0, f_y1))
                ops.append((0.0, 560.0, lambda: nc.vector.tensor_scalar(
                    y1[:, :], y1p[:, :], b1v[:, 0:1], 0.0,
                    ALU.add, ALU.max)))

                def f_scp(s0):
                    def fn():
                        for s in (s0, s0 + 1):
                            t = ca * CH + s
                            _mm(nc, scp[:, t:t + 1],
                                y1[:, s * BL:(s + 1) * BL],
                                W2T, start=True, stop=True)
                    return fn
                for s0 in range(0, CH, 2):
                    ops.append((420.0, 0.0, f_scp(s0)))
                return ops

            def iproj(c):
                XTc = XTtb[:, c * CG:(c + 1) * CG, :]
                prz = gip.tile([D, 2 * WG], f32, tag="girz")
                pn = gip.tile([D, WG], f32, tag="gin")
                _mm(nc, prz[:, 0:WG], WihT[:, 0:D], XTc,
                    start=True, stop=True)
                _mm(nc, prz[:, WG:2 * WG], WihT[:, D:2 * D], XTc,
                    start=True, stop=True)
                _mm(nc, pn[:, :], WihT[:, 2 * D:G3], XTc,
                    start=True, stop=True)
                return prz, pn

            cur = iproj(0)
            nxt = [None]
            for c in range(NCG):
                prz, pn = cur
                pn3 = pn.rearrange("p (s g) -> p s g", g=BL)
                for s in range(CG):
                    t = c * CG + s
                    r = gt.tile([D, BL], f32, tag="r")
                    oz = gt.tile([D, BL], f32, tag="oz")
                    if t > 0:
                        hprev = gruT3[:, t - 1, :]
                        # PSUM reads wait for every PE write to the same
                        # bank issued before them, so each sigmoid is
                        # issued immediately after its own gate matmul.
                        _mm(nc, prz[:, s * BL:(s + 1) * BL], WhhT[:, 0:D],
                            hprev, start=False, stop=True, skip=True)
                        nc.scalar.activation(r[:, :],
                                             prz[:, s * BL:(s + 1) * BL],
                                             SIG, bias=brz[:, 0:1])
                        hn = hnt
                        _mm(nc, hn[:, :], WhhT[:, 2 * D:G3], hprev,
                            start=True, stop=True)
                        tmp = gt.tile([D, BL], f32, tag="tmp")
                        nc.vector.scalar_tensor_tensor(
                            tmp[:, :], hn[:, :], b_hn, r[:, :],
                            ALU.add, ALU.mult)
                        _mm(nc, prz[:, WG + s * BL:WG + (s + 1) * BL],
                            WhhT[:, D:2 * D], hprev,
                            start=False, stop=True, skip=True)
                        nc.scalar.activation(
                            oz[:, :], prz[:, WG + s * BL:WG + (s + 1) * BL],
                            SIG, bias=negbz[:, 0:1], scale=-1.0)
                        nc.vector.tensor_add(pn3[:, s, :], tmp[:, :],
                                             pn3[:, s, :])
                        ozm1 = gt.tile([D, BL], f32, tag="ozm1")
                        nc.vector.tensor_scalar_add(ozm1[:, :], oz[:, :],
                                                    -1.0)
                        negw = gt.tile([D, BL], f32, tag="negw")
                        nc.gpsimd.tensor_mul(negw[:, :], ozm1[:, :], hprev)
                    else:
                        nc.scalar.activation(r[:, :],
                                             prz[:, s * BL:(s + 1) * BL],
                                             SIG, bias=brz[:, 0:1])
                        nc.scalar.activation(
                            oz[:, :], prz[:, WG + s * BL:WG + (s + 1) * BL],
                            SIG, bias=negbz[:, 0:1], scale=-1.0)
                        nc.vector.scalar_tensor_tensor(
                            pn3[:, s, :], r[:, :], b_hn, pn3[:, s, :],
                            ALU.mult, ALU.add)
                    n = gt.tile([D, BL], f32, tag="n")
                    nc.scalar.activation(n[:, :], pn3[:, s, :], TANH,
                                         bias=b_in)
                    if t > 0:
                        nm = gt.tile([D, BL], f32, tag="nm")
                        nc.vector.tensor_mul(nm[:, :], n[:, :], oz[:, :])
                        nc.vector.tensor_sub(gruT3[:, t, :], nm[:, :],
                                             negw[:, :])
                    else:
                        nc.vector.tensor_mul(gruT3[:, t, :], n[:, :],
                                             oz[:, :])
                    if s == 1 and c + 1 < NCG:
                        nxt[0] = iproj(c + 1)
                    pump()
                cur = nxt[0]
                if c % 2 == 1:
                    for op in attention_ops(c // 2):
                        opq.append(op)

            drain()

            # finalize masked history mean
            nc.scalar.copy(histF[:, :], histp[:, 0:BL])
            nc.vector.tensor_add(histF[:, :], histF[:, :], histp[:, BL:2 * BL])
            nc.scalar.copy(histT[:, :], histF[:, :])

            # masked softmax over t (b-major); last MLP layer ReLU'd w/ b2
            rawr = at.tile([BL, T], f32, tag="rawr")
            nc.scalar.activation(rawr[:, :], scp[:, :], AF.Relu,
                                 bias=b2col[:, 0:1])
            rawm = at.tile([BL, T], f32, tag="rawm")
            nc.vector.tensor_mul(rawm[:, :], rawr[:, :], maskS[:, :])
            mxn = at.tile([BL, 1], f32, tag="mxn")
            nc.vector.tensor_reduce(mxn[:, :], rawm[:, :],
                                    axis=mybir.AxisListType.X,
                                    op=ALU.max, negate=True)
            ex = at.tile([BL, T], f32, tag="ex")
            nc.scalar.activation(ex[:, :], rawr[:, :], AF.Exp, bias=mxn[:, 0:1])
            em = at.tile([BL, T], f32, tag="em")
            nc.vector.tensor_mul(em[:, :], ex[:, :], maskS[:, :])
            sm = at.tile([BL, 1], f32, tag="sm")
            nc.vector.tensor_reduce(sm[:, :], em[:, :],
                                    axis=mybir.AxisListType.X, op=ALU.add)
            rs = at.tile([BL, 1], f32, tag="rs")
            nc.vector.reciprocal(rs[:, :], sm[:, :])
            nc.vector.tensor_scalar_mul(wgt[:, :], em[:, :], rs[:, 0:1])

        wgt_bf = pp.tile([BL, T], bf16)
        nc.scalar.copy(wgt_bf[:, :], wgt[:, :])

        # ---------------- AUGRU ----------------------------------------
        hA = [None]
        with tc.tile_pool(name="axp", bufs=2, space="PSUM") as axp, \
             tc.tile_pool(name="abp", bufs=2, space="PSUM") as abp, \
             tc.tile_pool(name="ut", bufs=10) as ut:

            def aug_prep(c):
                gc = gruT[:, c * WG:(c + 1) * WG]
                pru = axp.tile([D, 2 * WG], f32, tag="pxru")
                pxh = axp.tile([D, WG], f32, tag="pxh")
                repp = abp.tile([CG, CG * BL], f32, tag="repp")
                masked = ut.tile([CG, CG * BL], bf16, tag="maskedw")
                pab = abp.tile([D, WG], f32, tag="pab")
                ops = []
                ops.append((420.0, 0.0, lambda: _mm(
                    nc, pru[:, 0:WG], WrxT, gc, start=True, stop=True)))
                ops.append((420.0, 0.0, lambda: _mm(
                    nc, pru[:, WG:2 * WG], WuxT, gc, start=True, stop=True)))
                ops.append((420.0, 0.0, lambda: _mm(
                    nc, pxh[:, :], WaxT, gc, start=True, stop=True)))

                def f_rep():
                    _mm(nc, repp[:, :], wgt_bf[:, c * CG:(c + 1) * CG],
                        identB4, start=True, stop=True)
                    nc.vector.tensor_mul(masked[:, :], repp[:, :],
                                         blkmask[:, :])
                ops.append((230.0, 450.0, f_rep))
                ops.append((330.0, 0.0, lambda: _mm(
                    nc, pab[:, :], ones4, masked, start=True, stop=True)))
                return (pru, pxh, pab), ops

            opq.clear()
            cur, ops0 = aug_prep(0)
            for _, _, fn in ops0:
                fn()
            nxt = [None]
            for c in range(NCG):
                pru, pxh, pab = cur
                pxh3 = pxh.rearrange("p (s g) -> p s g", g=BL)
                for s in range(CG):
                    t = c * CG + s
                    hAg = hA[0]
                    r = ut.tile([D, BL], f32, tag="ar")
                    u = ut.tile([D, BL], f32, tag="au")
                    if t > 0:
                        _mm(nc, pru[:, s * BL:(s + 1) * BL], WrhT, hAg,
                            start=False, stop=True, skip=True)
                        nc.scalar.activation(r[:, :],
                                             pru[:, s * BL:(s + 1) * BL],
                                             SIG, bias=ab_r)
                        _mm(nc, pru[:, WG + s * BL:WG + (s + 1) * BL],
                            WuhT, hAg, start=False, stop=True, skip=True)
                    else:
                        nc.scalar.activation(r[:, :],
                                             pru[:, s * BL:(s + 1) * BL],
                                             SIG, bias=ab_r)
                    nc.scalar.activation(u[:, :],
                                         pru[:, WG + s * BL:WG + (s + 1) * BL],
                                         SIG, bias=ab_u)
                    if t > 0:
                        hr = ut.tile([D, BL], bf16, tag="ahr")
                        nc.vector.tensor_mul(hr[:, :], hAg, r[:, :])
                        _mm(nc, pxh3[:, s, :], WahT, hr,
                            start=False, stop=True, skip=True)
                    up = ut.tile([D, BL], f32, tag="aup")
                    nc.vector.tensor_mul(up[:, :],
                                         pab[:, s * BL:(s + 1) * BL], u[:, :])
                    hh = ut.tile([D, BL], f32, tag="ahh")
                    nc.scalar.activation(hh[:, :], pxh3[:, s, :], TANH,
                                         bias=ab_h)
                    hnew = hp.tile([D, BL], bf16, tag="hA")
                    if t > 0:
                        dd = ut.tile([D, BL], f32, tag="add")
                        nc.vector.tensor_sub(dd[:, :], hh[:, :], hAg)
                        ud = ut.tile([D, BL], f32, tag="aud")
                        nc.vector.tensor_mul(ud[:, :], up[:, :], dd[:, :])
                        nc.vector.tensor_add(hnew[:, :], hAg, ud[:, :])
                    else:
                        nc.vector.tensor_mul(hnew[:, :], up[:, :], hh[:, :])
                    hA[0] = hnew
                    if s == 0 and c + 1 < NCG:
                        prep, ops = aug_prep(c + 1)
                        nxt[0] = prep
                        for op in ops:
                            opq.append(op)
                    pump(pe_budget=900.0, vec_budget=800.0)
                while opq:
                    _, _, fn = opq.popleft()
                    fn()
                cur = nxt[0]

        # ---------------- output layer ---------------------------------
        with tc.tile_pool(name="ops", bufs=1, space="PSUM") as ops_p, \
             tc.tile_pool(name="ot", bufs=1) as ot:
            ih = ot.tile([D, BL], bf16)
            nc.vector.tensor_mul(ih[:, :], qT[:, :], histF[:, :])
            po = ops_p.tile([1, BL], f32)
            pieces = [userT, qT, histT, ih, hA[0]]
            for g, piece in enumerate(pieces):
                _mm(nc, po[:, :], outWT[:, g:g + 1], piece,
                    start=(g == 0), stop=(g == 4))
            outs = ot.tile([1, BL], f32)
            nc.scalar.activation(outs[:, :], po[:, :], AF.Identity,
                                 bias=outb[:, 0:1])
            nc.sync.dma_start(outd[:, :], outs[:, :])

    nc.finalize()
    return nc


_NC = None


def _get_nc():
    global _NC
    if _NC is None:
        _NC = build_nc()
    return _NC


def make_in_maps(inputs):
    """Slice full inputs into per-core input maps (host-side layout only)."""
    f = {k: np.asarray(v) for k, v in inputs.items()}
    WihT = np.ascontiguousarray(f["gru_Wih"].T)          # (128, 384)
    WhhT = np.ascontiguousarray(f["gru_Whh"].T)
    bihT = np.ascontiguousarray(f["gru_bih"].reshape(3, D).T)  # (128, 3)
    bhhT = np.ascontiguousarray(f["gru_bhh"].reshape(3, D).T)
    W0T = np.ascontiguousarray(
        f["attn_W0"].T.reshape(4, D, 80).transpose(1, 0, 2).reshape(D, 320))
    b0 = np.ascontiguousarray(f["attn_b0"].reshape(80, 1))
    W1T = np.ascontiguousarray(f["attn_W1"].T)           # (80, 40)
    b1 = np.ascontiguousarray(f["attn_b1"].reshape(40, 1))
    W2T = np.ascontiguousarray(f["attn_W2"].T)           # (40, 1)
    b2 = f["attn_b2"].reshape(1, 1)
    augW = np.concatenate(
        [np.ascontiguousarray(f[k][:, p * D:(p + 1) * D].T)
         for k in ("aug_Wr", "aug_Wu", "aug_Wh") for p in (0, 1)],
        axis=1)                                          # (128, 768)
    augb = np.stack([f["aug_br"], f["aug_bu"], f["aug_bh"]], axis=1)  # (128,3)
    outWT = np.ascontiguousarray(f["out_W"].reshape(5, D).T)          # (128,5)
    outb = f["out_b"].reshape(1, 1)

    shared_bf = dict(WihT=WihT, WhhT=WhhT, W0T=W0T, W1T=W1T, W2T=W2T,
                     augW=augW, outWT=outWT)
    shared = dict(bihT=bihT, bhhT=bhhT, b0=b0, b1=b1, b2=b2, augb=augb,
                  outb=outb)
    shared = {k: np.ascontiguousarray(v.astype(np.float32)) for k, v in
              shared.items()}
    shared.update({k: np.ascontiguousarray(v.astype(BF)) for k, v in
                   shared_bf.items()})
    blk = np.zeros((CG, CG * BL), dtype=BF)
    for s in range(CG):
        blk[s, s * BL:(s + 1) * BL] = 1
    shared["blkmask"] = blk

    in_maps = []
    for c in range(NCORES):
        s = slice(c * BL, (c + 1) * BL)
        m = dict(shared)
        m["hist"] = np.ascontiguousarray(
            f["item_historical_embedding"][s].astype(BF))
        m["item"] = np.ascontiguousarray(f["item_embedding"][s].astype(np.float32))
        m["user"] = np.ascontiguousarray(f["user_embedding"][s].astype(np.float32))
        m["maskd"] = np.ascontiguousarray(f["mask"][s].astype(np.float32))
        m["seqd"] = np.ascontiguousarray(
            f["sequential_length"][s].reshape(BL, 1).astype(np.int32))
        in_maps.append(m)
    return in_maps


def kernel(**inputs) -> np.ndarray:
    nc = _get_nc()
    in_maps = make_in_maps(inputs)
    res = run_bass_kernel_spmd(nc, in_maps, list(range(NCORES)))
    return np.concatenate(
        [np.asarray(res.results[c]["out"]).reshape(BL) for c in range(NCORES)])


# revision 80
# speedup vs baseline: 1.0871x; 1.0187x over previous
"""DIEN (GRU + attention + AUGRU) Trainium2 kernel.

Data-parallel over 8 NeuronCores: each core handles a batch slice of 64.
All on-chip state is feature-major (feature dim on SBUF partitions, batch
on the free dim), so the two sequential recurrences need no per-step
transposes.

The kernel is latency-bound on the two serial recurrences, so the design
minimizes the per-step dependency chain:
  - Per-gate sigmoids read gate PSUM directly with the bias folded into
    the ACT instruction (bias ap / scale=-1 for the update gate, which is
    consumed as oz = 1-z), so no PSUM bias-add ops exist at all.
  - h-update tail is 2 vector ops: hnew = n*oz - negw, where
    negw = (oz-1)*hprev is one fused STT on the (idle) GPSIMD engine,
    issued in the tanh shadow.
  - PE issue order per step is mm_r, mm_hn, mm_z so the sigmoid and the
    n-path start as early as possible.
  - Attention (interleaved with the GRU), AUGRU input projections, the
    X^T build second half, and the masked history sum are spread across
    the step loop via a cost-budgeted op queue so no chunk-boundary PE
    burst delays the chain.
  - AUGRU attention-weight broadcast uses a block-diagonal 2-matmul
    construction per 4-step chunk (replicate rows via identity-bank
    matmul, mask to block-diagonal, ones-matmul broadcast).
  - Masked steps have weight 0 (u'=0, h unchanged), so the final AUGRU
    state IS the gathered aug_out[b, len-1] - no gather needed.
  - Matmuls are bf16 (f32 PSUM accumulation); the masked history mean
    and softmax stay f32.
"""

import sys

if "/opt/trn_rl_repo" not in sys.path:
    sys.path.insert(0, "/opt/trn_rl_repo")

from collections import deque
from contextlib import ExitStack

import ml_dtypes
import numpy as np

import concourse.bacc as bacc
import concourse.bass as bass
import concourse.mybir as mybir
import concourse.tile as tile
from concourse.bass_utils import run_bass_kernel_spmd
from concourse.masks import make_identity

f32 = mybir.dt.float32
bf16 = mybir.dt.bfloat16
i32 = mybir.dt.int32
AF = mybir.ActivationFunctionType
ALU = mybir.AluOpType
BF = ml_dtypes.bfloat16

NCORES = 8
B, T, D = 512, 200, 128
BL = B // NCORES          # 64 batch rows per core
CG = 4                    # GRU/AUGRU psum chunk: steps per chunk
NCG = T // CG             # 50
WG = CG * BL              # 256
CH = 8                    # attention chunk: steps per chunk
NCHUNK = T // CH          # 25
W = CH * BL               # 512
G3 = 3 * D


def _mm(nc, out, lhsT, rhs, start, stop, skip=False):
    nc.tensor.matmul(out, lhsT, rhs, start=start, stop=stop,
                     skip_group_check=skip)


def build_nc():
    nc = bacc.Bacc("TRN2", target_bir_lowering=False)

    hist = nc.declare_dram_parameter("hist", [BL, T, D], bf16, isOutput=False)
    item = nc.declare_dram_parameter("item", [BL, D], f32, isOutput=False)
    user = nc.declare_dram_parameter("user", [BL, D], f32, isOutput=False)
    maskd = nc.declare_dram_parameter("maskd", [BL, T], f32, isOutput=False)
    seqd = nc.declare_dram_parameter("seqd", [BL, 1], i32, isOutput=False)
    WihTd = nc.declare_dram_parameter("WihT", [D, G3], bf16, isOutput=False)
    WhhTd = nc.declare_dram_parameter("WhhT", [D, G3], bf16, isOutput=False)
    bihTd = nc.declare_dram_parameter("bihT", [D, 3], f32, isOutput=False)
    bhhTd = nc.declare_dram_parameter("bhhT", [D, 3], f32, isOutput=False)
    W0Td = nc.declare_dram_parameter("W0T", [D, 320], bf16, isOutput=False)
    b0d = nc.declare_dram_parameter("b0", [80, 1], f32, isOutput=False)
    W1Td = nc.declare_dram_parameter("W1T", [80, 40], bf16, isOutput=False)
    b1d = nc.declare_dram_parameter("b1", [40, 1], f32, isOutput=False)
    W2Td = nc.declare_dram_parameter("W2T", [40, 1], bf16, isOutput=False)
    b2d = nc.declare_dram_parameter("b2", [1, 1], f32, isOutput=False)
    augWd = nc.declare_dram_parameter("augW", [D, 6 * D], bf16, isOutput=False)
    augbd = nc.declare_dram_parameter("augb", [D, 3], f32, isOutput=False)
    outWTd = nc.declare_dram_parameter("outWT", [D, 5], bf16, isOutput=False)
    outbd = nc.declare_dram_parameter("outb", [1, 1], f32, isOutput=False)
    blkd = nc.declare_dram_parameter("blkmask", [CG, CG * BL], bf16,
                                     isOutput=False)
    outd = nc.declare_dram_parameter("out", [1, BL], f32, isOutput=True)

    with tile.TileContext(nc) as tc, ExitStack() as ctx:
        big = ctx.enter_context(tc.tile_pool(name="big", bufs=1))
        wp = ctx.enter_context(tc.tile_pool(name="wp", bufs=1))
        pp = ctx.enter_context(tc.tile_pool(name="pp", bufs=1))
        hp = ctx.enter_context(tc.tile_pool(name="hp", bufs=8))

        XT = big.tile([D, T * BL], bf16)
        gruT = big.tile([D, T * BL], bf16)
        # natural-layout history tiles kept resident so the masked history
        # sum matmuls can run interleaved with the GRU loop
        xk0 = big.tile([128, BL * D], bf16)   # t in [0,128)
        xk1 = big.tile([72, BL * D], bf16)    # t in [128,200)

        ident = pp.tile([128, 128], f32)
        make_identity(nc, ident)
        identB = pp.tile([BL, BL], bf16)
        make_identity(nc, identB)
        identB128 = pp.tile([128, 128], bf16)
        make_identity(nc, identB128)
        # 4 copies of identB side by side (AUGRU weight-broadcast trick)
        identB4 = pp.tile([BL, CG * BL], bf16)
        for j in range(CG):
            nc.scalar.copy(identB4[:, j * BL:(j + 1) * BL], identB[:, :])
        # block-diagonal mask [s, s*BL:(s+1)*BL] = 1 (host-built constant;
        # sub-partition-offset memsets are not addressable)
        blkmask = pp.tile([CG, CG * BL], bf16)
        nc.sync.dma_start(blkmask[:, :], blkd[:, :])
        ones4 = pp.tile([CG, 128], bf16)
        nc.vector.memset(ones4[:, :], 1.0)

        # ------------- weights (bf16 arrive pre-converted via DMA) -------
        WihT = wp.tile([D, G3], bf16)
        nc.sync.dma_start(WihT[:, :], WihTd[:, :])
        WhhT = wp.tile([D, G3], bf16)
        nc.sync.dma_start(WhhT[:, :], WhhTd[:, :])
        W0T = wp.tile([D, 320], bf16)
        nc.sync.dma_start(W0T[:, :], W0Td[:, :])
        W1T = wp.tile([80, 40], bf16)
        nc.sync.dma_start(W1T[:, :], W1Td[:, :])
        W2T = wp.tile([40, 1], bf16)
        nc.sync.dma_start(W2T[:, :], W2Td[:, :])
        augW = wp.tile([D, 6 * D], bf16)
        nc.sync.dma_start(augW[:, :], augWd[:, :])
        outWT = wp.tile([D, 5], bf16)
        nc.sync.dma_start(outWT[:, :], outWTd[:, :])
        WrhT, WrxT = augW[:, 0:D], augW[:, D:2 * D]
        WuhT, WuxT = augW[:, 2 * D:3 * D], augW[:, 3 * D:4 * D]
        WahT, WaxT = augW[:, 4 * D:5 * D], augW[:, 5 * D:6 * D]

        # attention layer-0 decomposition: W0·[f;q;f*q;q-f] =
        # (W0f-W0d)·f + W0p·(f*q) + (W0q+W0d)·q; the q-term is constant
        # across t per batch row and is preloaded into PSUM per chunk.
        AT = wp.tile([D, 80], bf16)
        nc.vector.tensor_sub(AT[:, :], W0T[:, 0:80], W0T[:, 240:320])
        BqT = wp.tile([D, 80], bf16)
        nc.vector.tensor_add(BqT[:, :], W0T[:, 80:160], W0T[:, 240:320])
        W0pT = W0T[:, 160:240]

        bihT = wp.tile([D, 3], f32)
        nc.sync.dma_start(bihT[:, :], bihTd[:, :])
        bhhT = wp.tile([D, 3], f32)
        nc.sync.dma_start(bhhT[:, :], bhhTd[:, :])
        brz = wp.tile([D, 2], f32)
        nc.vector.tensor_add(brz[:, :], bihT[:, 0:2], bhhT[:, 0:2])
        negbz = wp.tile([D, 1], f32)
        nc.vector.tensor_scalar_mul(negbz[:, :], brz[:, 1:2], -1.0)
        b_hn, b_in = bhhT[:, 2:3], bihT[:, 2:3]

        b0v = wp.tile([80, 1], f32)
        nc.sync.dma_start(b0v[:, :], b0d[:, :])
        b1v = wp.tile([40, 1], f32)
        nc.sync.dma_start(b1v[:, :], b1d[:, :])
        b2v = wp.tile([1, 1], f32)
        nc.sync.dma_start(b2v[:, :], b2d[:, :])
        augb = wp.tile([D, 3], f32)
        nc.sync.dma_start(augb[:, :], augbd[:, :])
        ab_r, ab_u, ab_h = augb[:, 0:1], augb[:, 1:2], augb[:, 2:3]
        outb = wp.tile([1, 1], f32)
        nc.sync.dma_start(outb[:, :], outbd[:, :])

        ones_f = wp.tile([1, 128], f32)
        nc.vector.memset(ones_f[:, :], 1.0)
        onesDB = wp.tile([D, BL], f32)
        nc.vector.memset(onesDB[:, :], 1.0)

        # ---------------- small preprocessing ---------------------------
        histT = pp.tile([D, BL], bf16)
        qT = pp.tile([D, BL], bf16)
        userT = pp.tile([D, BL], bf16)
        qRep = pp.tile([D, W], bf16)
        maskS = pp.tile([BL, T], f32)
        b2col = pp.tile([BL, 1], f32)
        histF = pp.tile([D, BL], f32)
        mst0 = pp.tile([128, BL], bf16)
        mst1 = pp.tile([72, BL], bf16)

        XT3 = XT.rearrange("p (t b) -> p t b", b=BL)

        with tc.tile_pool(name="pre", bufs=4) as pre, \
             tc.tile_pool(name="pps", bufs=2, space="PSUM") as pps:
            nc.sync.dma_start(maskS[:, :], maskd[:, :])
            seqi = pre.tile([BL, 1], i32)
            nc.sync.dma_start(seqi[:, :], seqd[:, :])
            seqf = pre.tile([BL, 1], f32)
            nc.vector.tensor_copy(seqf[:, :], seqi[:, :])
            rsec = pre.tile([BL, 1], f32)
            nc.vector.reciprocal(rsec[:, :], seqf[:, :])
            maskSc = pre.tile([BL, T], f32)
            nc.vector.tensor_scalar_mul(maskSc[:, :], maskS[:, :], rsec[:, 0:1])

            mstp0 = pps.tile([128, BL], f32, tag="mstp")
            nc.tensor.transpose(mstp0[:, :], maskSc[:, 0:128], ident[0:BL, 0:BL])
            nc.scalar.copy(mst0[:, :], mstp0[:, :])
            mstp1 = pps.tile([128, BL], f32, tag="mstp")
            nc.tensor.transpose(mstp1[0:72, :], maskSc[:, 128:200], ident[0:BL, 0:BL])
            nc.scalar.copy(mst1[:, :], mstp1[0:72, :])

            itn = pre.tile([BL, D], f32, tag="itn")
            nc.sync.dma_start(itn[:, :], item[:, :])
            itp = pps.tile([D, BL], f32, tag="small_t")
            nc.tensor.transpose(itp[:, :], itn[:, :], ident[0:BL, 0:BL])
            nc.scalar.copy(qT[:, :], itp[:, :])
            usn = pre.tile([BL, D], f32, tag="itn")
            nc.sync.dma_start(usn[:, :], user[:, :])
            usp = pps.tile([D, BL], f32, tag="small_t")
            nc.tensor.transpose(usp[:, :], usn[:, :], ident[0:BL, 0:BL])
            nc.scalar.copy(userT[:, :], usp[:, :])
            b2p = pps.tile([BL, 1], f32, tag="small_t")
            _mm(nc, b2p[:, :], ones_f[0:1, 0:BL], b2v, start=True, stop=True)
            nc.scalar.copy(b2col[:, :], b2p[:, :])
            for s in range(CH):
                nc.scalar.copy(qRep[:, s * BL:(s + 1) * BL], qT[:, :])


            # X^T t<64: staged DMA + PE transpose upfront (all the GRU
            # needs to start). X^T t in [64,200): hardware transpose-DMAs
            # on the idle sync queue, overlapped with the GRU loop (first
            # needed at chunk 16, ~150us after they start). xk0/xk1
            # natural-layout copies feed the masked-history-sum matmuls.
            xk03 = xk0.rearrange("p (b d) -> p b d", d=D)
            xk13 = xk1.rearrange("p (b d) -> p b d", d=D)
            for b8 in range(0, BL, 8):
                nc.sync.dma_start(
                    xk03[:, b8:b8 + 8, :],
                    hist[b8:b8 + 8, 0:128, :].rearrange("b t d -> t b d"))
            for b4 in range(0, BL, 4):
                xtp = pps.tile([D, 4 * 64], bf16, tag="xtp")
                for j in range(4):
                    nc.tensor.transpose(
                        xtp[:, j * 64:(j + 1) * 64],
                        xk0[0:64, (b4 + j) * D:(b4 + j + 1) * D],
                        identB128[0:64, 0:64])
                nc.vector.tensor_copy(
                    XT3[:, 0:64, b4:b4 + 4],
                    xtp.rearrange("p (b t) -> p t b", b=4))
            for b8 in range(0, BL, 8):
                nc.sync.dma_start(
                    xk13[:, b8:b8 + 8, :],
                    hist[b8:b8 + 8, 128:200, :].rearrange("b t d -> t b d"))

        # ---------------- GRU + interleaved deferred work ---------------
        gruT3 = gruT.rearrange("p (t b) -> p t b", b=BL)
        SIG, TANH = AF.Sigmoid, AF.Tanh
        wgt = pp.tile([BL, T], f32)

        # cost-budgeted deferred-op queues: (pe_ns, vec_ns, closure).
        # opq is high priority (attention / AUGRU prep), opq_lo fills the
        # remaining budget (X^T second half, masked history sums).
        opq = deque()
        opq_lo = deque()

        def pump(pe_budget=800.0, vec_budget=800.0):
            pe_left, vec_left = pe_budget, vec_budget
            while opq:
                pe_c, vec_c, fn = opq[0]
                if pe_c > pe_left or vec_c > vec_left:
                    break
                opq.popleft()
                fn()
                pe_left -= pe_c
                vec_left -= vec_c
            # exactly one low-priority piece per step, unconditionally:
            # X^T second-half columns MUST all be written (program order)
            # before the chunk-32 input projection reads them, and the
            # single-buffered staging slot forbids more than one per step
            if opq_lo:
                opq_lo.popleft()[2]()

        def drain():
            for q in (opq, opq_lo):
                while q:
                    q.popleft()[2]()

        with tc.tile_pool(name="gip", bufs=2, space="PSUM") as gip, \
             tc.tile_pool(name="gt", bufs=10) as gt, \
             tc.tile_pool(name="aps", bufs=1, space="PSUM") as aps, \
             tc.tile_pool(name="msc", bufs=1, space="PSUM") as msc, \
             tc.tile_pool(name="xtb", bufs=1, space="PSUM") as xtb, \
             tc.tile_pool(name="at", bufs=3) as at:
            # one bank shared by the recurrent n-gate psum, the masked
            # history sums and the attention score columns
            smalls = msc.tile([D, 512], f32, tag="smalls")
            hnt = smalls[:, 0:BL]
            histp = smalls[:, BL:3 * BL]        # [first half | second half]
            scp = smalls[0:BL, 312:512]
            xtp2s = xtb.tile([D, 256], bf16, tag="xtp2")

            # deferred X^T build pieces (pairs of batch rows per piece,
            # staged through one psum bank) + masked history sums.
            # Deadlines (guaranteed by exactly-1-piece-per-step pumping):
            # t in [64,128) pieces done by step 32 (needed at chunk 16),
            # t in [128,200) by step 64 (needed at chunk 32).
            def xt_pair(src, p0, t0, tn, b):
                def fn():
                    for j in (0, 1):
                        nc.tensor.transpose(
                            xtp2s[:, j * 128:j * 128 + tn],
                            src[p0:p0 + tn, (b + j) * D:(b + j + 1) * D],
                            identB128[p0:p0 + tn, p0:p0 + tn])
                    nc.vector.tensor_copy(
                        XT3[:, t0:t0 + tn, b:b + 2],
                        xtp2s.rearrange("p (b t) -> p t b", b=2)[:, 0:tn, :])
                return (470.0, 300.0, fn)

            def hist_piece(b):
                def fn():
                    _mm(nc, histp[:, b:b + 1],
                        xk0[:, b * D:(b + 1) * D], mst0[:, b:b + 1],
                        start=True, stop=True)
                    _mm(nc, histp[:, BL + b:BL + b + 1],
                        xk1[:, b * D:(b + 1) * D],
                        mst1[:, b:b + 1], start=True, stop=True)
                return (400.0, 0.0, fn)

            for b in range(0, BL, 2):
                opq_lo.append(xt_pair(xk0, 64, 64, 64, b))
            for b in range(0, BL, 2):
                opq_lo.append(xt_pair(xk1, 0, 128, 72, b))
            for b in range(BL):
                opq_lo.append(hist_piece(b))

            # attention for 8-step chunk ca, split into budgeted pieces
            def attention_ops(ca):
                gc = gruT[:, ca * W:(ca + 1) * W]
                r2 = at.tile([D, W], bf16, tag="r2")
                y0p = aps.tile([80, W], f32, tag="y0")
                y0 = at.tile([80, W], bf16, tag="y0s")
                y1p = aps.tile([40, W], f32, tag="y1")
                y1 = at.tile([40, W], bf16, tag="y1s")
                ops = []
                H = W // 2
                ops.append((0.0, 360.0, lambda: nc.vector.tensor_mul(
                    r2[:, 0:H], gc[:, 0:H], qRep[:, 0:H])))
                ops.append((0.0, 360.0, lambda: nc.vector.tensor_mul(
                    r2[:, H:W], gc[:, H:W], qRep[:, H:W])))
                ops.append((640.0, 0.0, lambda: _mm(
                    nc, y0p[:, :], AT, gc, start=True, stop=False)))
                ops.append((640.0, 0.0, lambda: _mm(
                    nc, y0p[:, :], BqT, qRep, start=False, stop=False)))
                ops.append((640.0, 0.0, lambda: _mm(
                    nc, y0p[:, :], W0pT, r2, start=False, stop=True)))
                ops.append((0.0, 440.0, lambda: nc.vector.tensor_scalar(
                    y0[:, 0:H], y0p[:, 0:H], b0v[:, 0:1], 0.0,
                    ALU.add, ALU.max)))
                ops.append((0.0, 440.0, lambda: nc.vector.tensor_scalar(
                    y0[:, H:W], y0p[:, H:W], b0v[:, 0:1], 0.0,
                    ALU.add, ALU.max)))

                def f_y1():
                    _mm(nc, y1p[:, :], W1T, y0, start=True, stop=True)
                ops.append((320.0, 0.0, f_y1))
                ops.append((0.0, 560.0, lambda: nc.vector.tensor_scalar(
                    y1[:, :], y1p[:, :], b1v[:, 0:1], 0.0,
                    ALU.add, ALU.max)))

                def f_scp(s0):
                    def fn():
                        for s in (s0, s0 + 1):
                            t = ca * CH + s
                            _mm(nc, scp[:, t:t + 1],
                                y1[:, s * BL:(s + 1) * BL],
                                W2T, start=True, stop=True)
                    return fn
                for s0 in range(0, CH, 2):
                    ops.append((420.0, 0.0, f_scp(s0)))
                return ops

            def iproj(c):
                XTc = XT[:, c * WG:(c + 1) * WG]
                prz = gip.tile([D, 2 * WG], f32, tag="girz")
                pn = gip.tile([D, WG], f32, tag="gin")
                _mm(nc, prz[:, 0:WG], WihT[:, 0:D], XTc,
                    start=True, stop=True)
                _mm(nc, prz[:, WG:2 * WG], WihT[:, D:2 * D], XTc,
                    start=True, stop=True)
                _mm(nc, pn[:, :], WihT[:, 2 * D:G3], XTc,
                    start=True, stop=True)
                return prz, pn

            cur = iproj(0)
            nxt = [None]
            for c in range(NCG):
                prz, pn = cur
                pn3 = pn.rearrange("p (s g) -> p s g", g=BL)
                for s in range(CG):
                    t = c * CG + s
                    r = gt.tile([D, BL], f32, tag="r")
                    oz = gt.tile([D, BL], f32, tag="oz")
                    if t > 0:
                        hprev = gruT3[:, t - 1, :]
                        # PSUM reads wait for every PE write to the same
                        # bank issued before them, so each sigmoid is
                        # issued immediately after its own gate matmul.
                        _mm(nc, prz[:, s * BL:(s + 1) * BL], WhhT[:, 0:D],
                            hprev, start=False, stop=True, skip=True)
                        nc.scalar.activation(r[:, :],
                                             prz[:, s * BL:(s + 1) * BL],
                                             SIG, bias=brz[:, 0:1])
                        hn = hnt
                        _mm(nc, hn[:, :], WhhT[:, 2 * D:G3], hprev,
                            start=True, stop=True)
                        tmp = gt.tile([D, BL], f32, tag="tmp")
                        nc.vector.scalar_tensor_tensor(
                            tmp[:, :], hn[:, :], b_hn, r[:, :],
                            ALU.add, ALU.mult)
                        _mm(nc, prz[:, WG + s * BL:WG + (s + 1) * BL],
                            WhhT[:, D:2 * D], hprev,
                            start=False, stop=True, skip=True)
                        nc.scalar.activation(
                            oz[:, :], prz[:, WG + s * BL:WG + (s + 1) * BL],
                            SIG, bias=negbz[:, 0:1], scale=-1.0)
                        nc.vector.tensor_add(pn3[:, s, :], tmp[:, :],
                                             pn3[:, s, :])
                        ozm1 = gt.tile([D, BL], f32, tag="ozm1")
                        nc.vector.tensor_scalar_add(ozm1[:, :], oz[:, :],
                                                    -1.0)
                        negw = gt.tile([D, BL], f32, tag="negw")
                        nc.gpsimd.tensor_mul(negw[:, :], ozm1[:, :], hprev)
                    else:
                        nc.scalar.activation(r[:, :],
                                             prz[:, s * BL:(s + 1) * BL],
                                             SIG, bias=brz[:, 0:1])
                        nc.scalar.activation(
                            oz[:, :], prz[:, WG + s * BL:WG + (s + 1) * BL],
                            SIG, bias=negbz[:, 0:1], scale=-1.0)
                        nc.vector.scalar_tensor_tensor(
                            pn3[:, s, :], r[:, :], b_hn, pn3[:, s, :],
                            ALU.mult, ALU.add)
                    n = gt.tile([D, BL], f32, tag="n")
                    nc.scalar.activation(n[:, :], pn3[:, s, :], TANH,
                                         bias=b_in)
                    if t > 0:
                        nm = gt.tile([D, BL], f32, tag="nm")
                        nc.vector.tensor_mul(nm[:, :], n[:, :], oz[:, :])
                        nc.vector.tensor_sub(gruT3[:, t, :], nm[:, :],
                                             negw[:, :])
                    else:
                        nc.vector.tensor_mul(gruT3[:, t, :], n[:, :],
                                             oz[:, :])
                    if s == 1 and c + 1 < NCG:
                        nxt[0] = iproj(c + 1)
                    pump()
                cur = nxt[0]
                if c % 2 == 1:
                    for op in attention_ops(c // 2):
                        opq.append(op)

            drain()

            # finalize masked history mean
            nc.scalar.copy(histF[:, :], histp[:, 0:BL])
            nc.vector.tensor_add(histF[:, :], histF[:, :], histp[:, BL:2 * BL])
            nc.scalar.copy(histT[:, :], histF[:, :])

            # masked softmax over t (b-major); last MLP layer ReLU'd w/ b2
            rawr = at.tile([BL, T], f32, tag="rawr")
            nc.scalar.activation(rawr[:, :], scp[:, :], AF.Relu,
                                 bias=b2col[:, 0:1])
            rawm = at.tile([BL, T], f32, tag="rawm")
            nc.vector.tensor_mul(rawm[:, :], rawr[:, :], maskS[:, :])
            mxn = at.tile([BL, 1], f32, tag="mxn")
            nc.vector.tensor_reduce(mxn[:, :], rawm[:, :],
                                    axis=mybir.AxisListType.X,
                                    op=ALU.max, negate=True)
            ex = at.tile([BL, T], f32, tag="ex")
            nc.scalar.activation(ex[:, :], rawr[:, :], AF.Exp, bias=mxn[:, 0:1])
            em = at.tile([BL, T], f32, tag="em")
            nc.vector.tensor_mul(em[:, :], ex[:, :], maskS[:, :])
            sm = at.tile([BL, 1], f32, tag="sm")
            nc.vector.tensor_reduce(sm[:, :], em[:, :],
                                    axis=mybir.AxisListType.X, op=ALU.add)
            rs = at.tile([BL, 1], f32, tag="rs")
            nc.vector.reciprocal(rs[:, :], sm[:, :])
            nc.vector.tensor_scalar_mul(wgt[:, :], em[:, :], rs[:, 0:1])

        wgt_bf = pp.tile([BL, T], bf16)
        nc.scalar.copy(wgt_bf[:, :], wgt[:, :])

        # ---------------- AUGRU ----------------------------------------
        hA = [None]
        with tc.tile_pool(name="axp", bufs=2, space="PSUM") as axp, \
             tc.tile_pool(name="abp", bufs=2, space="PSUM") as abp, \
             tc.tile_pool(name="ut", bufs=10) as ut:

            def aug_prep(c):
                gc = gruT[:, c * WG:(c + 1) * WG]
                pru = axp.tile([D, 2 * WG], f32, tag="pxru")
                pxh = axp.tile([D, WG], f32, tag="pxh")
                repp = abp.tile([CG, CG * BL], f32, tag="repp")
                masked = ut.tile([CG, CG * BL], bf16, tag="maskedw")
                pab = abp.tile([D, WG], f32, tag="pab")
                ops = []
                ops.append((420.0, 0.0, lambda: _mm(
                    nc, pru[:, 0:WG], WrxT, gc, start=True, stop=True)))
                ops.append((420.0, 0.0, lambda: _mm(
                    nc, pru[:, WG:2 * WG], WuxT, gc, start=True, stop=True)))
                ops.append((420.0, 0.0, lambda: _mm(
                    nc, pxh[:, :], WaxT, gc, start=True, stop=True)))

                def f_rep():
                    _mm(nc, repp[:, :], wgt_bf[:, c * CG:(c + 1) * CG],
                        identB4, start=True, stop=True)
                    nc.vector.tensor_mul(masked[:, :], repp[:, :],
                                         blkmask[:, :])
                ops.append((230.0, 450.0, f_rep))
                ops.append((330.0, 0.0, lambda: _mm(
                    nc, pab[:, :], ones4, masked, start=True, stop=True)))
                return (pru, pxh, pab), ops

            opq.clear()
            cur, ops0 = aug_prep(0)
            for _, _, fn in ops0:
                fn()
            nxt = [None]
            for c in range(NCG):
                pru, pxh, pab = cur
                pxh3 = pxh.rearrange("p (s g) -> p s g", g=BL)
                for s in range(CG):
                    t = c * CG + s
                    hAg = hA[0]
                    r = ut.tile([D, BL], f32, tag="ar")
                    u = ut.tile([D, BL], f32, tag="au")
                    if t > 0:
                        _mm(nc, pru[:, s * BL:(s + 1) * BL], WrhT, hAg,
                            start=False, stop=True, skip=True)
                        nc.scalar.activation(r[:, :],
                                             pru[:, s * BL:(s + 1) * BL],
                                             SIG, bias=ab_r)
                        _mm(nc, pru[:, WG + s * BL:WG + (s + 1) * BL],
                            WuhT, hAg, start=False, stop=True, skip=True)
                    else:
                        nc.scalar.activation(r[:, :],
                                             pru[:, s * BL:(s + 1) * BL],
                                             SIG, bias=ab_r)
                    nc.scalar.activation(u[:, :],
                                         pru[:, WG + s * BL:WG + (s + 1) * BL],
                                         SIG, bias=ab_u)
                    if t > 0:
                        hr = ut.tile([D, BL], bf16, tag="ahr")
                        nc.vector.tensor_mul(hr[:, :], hAg, r[:, :])
                        _mm(nc, pxh3[:, s, :], WahT, hr,
                            start=False, stop=True, skip=True)
                    up = ut.tile([D, BL], f32, tag="aup")
                    nc.vector.tensor_mul(up[:, :],
                                         pab[:, s * BL:(s + 1) * BL], u[:, :])
                    hh = ut.tile([D, BL], f32, tag="ahh")
                    nc.scalar.activation(hh[:, :], pxh3[:, s, :], TANH,
                                         bias=ab_h)
                    hnew = hp.tile([D, BL], bf16, tag="hA")
                    if t > 0:
                        dd = ut.tile([D, BL], f32, tag="add")
                        nc.vector.tensor_sub(dd[:, :], hh[:, :], hAg)
                        ud = ut.tile([D, BL], f32, tag="aud")
                        nc.vector.tensor_mul(ud[:, :], up[:, :], dd[:, :])
                        nc.vector.tensor_add(hnew[:, :], hAg, ud[:, :])
                    else:
                        nc.vector.tensor_mul(hnew[:, :], up[:, :], hh[:, :])
                    hA[0] = hnew
                    if s == 0 and c + 1 < NCG:
                        prep, ops = aug_prep(c + 1)
                        nxt[0] = prep
                        for op in ops:
                            opq.append(op)
                    pump(pe_budget=900.0, vec_budget=800.0)
                while opq:
                    _, _, fn = opq.popleft()
                    fn()
                cur = nxt[0]

        # ---------------- output layer ---------------------------------
        with tc.tile_pool(name="ops", bufs=1, space="PSUM") as ops_p, \
             tc.tile_pool(name="ot", bufs=1) as ot:
            ih = ot.tile([D, BL], bf16)
            nc.vector.tensor_mul(ih[:, :], qT[:, :], histF[:, :])
            po = ops_p.tile([1, BL], f32)
            pieces = [userT, qT, histT, ih, hA[0]]
            for g, piece in enumerate(pieces):
                _mm(nc, po[:, :], outWT[:, g:g + 1], piece,
                    start=(g == 0), stop=(g == 4))
            outs = ot.tile([1, BL], f32)
            nc.scalar.activation(outs[:, :], po[:, :], AF.Identity,
                                 bias=outb[:, 0:1])
            nc.sync.dma_start(outd[:, :], outs[:, :])

    nc.finalize()
    return nc


_NC = None


def _get_nc():
    global _NC
    if _NC is None:
        _NC = build_nc()
    return _NC


def make_in_maps(inputs):
    """Slice full inputs into per-core input maps (host-side layout only)."""
    f = {k: np.asarray(v) for k, v in inputs.items()}
    WihT = np.ascontiguousarray(f["gru_Wih"].T)          # (128, 384)
    WhhT = np.ascontiguousarray(f["gru_Whh"].T)
    bihT = np.ascontiguousarray(f["gru_bih"].reshape(3, D).T)  # (128, 3)
    bhhT = np.ascontiguousarray(f["gru_bhh"].reshape(3, D).T)
    W0T = np.ascontiguousarray(
        f["attn_W0"].T.reshape(4, D, 80).transpose(1, 0, 2).reshape(D, 320))
    b0 = np.ascontiguousarray(f["attn_b0"].reshape(80, 1))
    W1T = np.ascontiguousarray(f["attn_W1"].T)           # (80, 40)
    b1 = np.ascontiguousarray(f["attn_b1"].reshape(40, 1))
    W2T = np.ascontiguousarray(f["attn_W2"].T)           # (40, 1)
    b2 = f["attn_b2"].reshape(1, 1)
    augW = np.concatenate(
        [np.ascontiguousarray(f[k][:, p * D:(p + 1) * D].T)
         for k in ("aug_Wr", "aug_Wu", "aug_Wh") for p in (0, 1)],
        axis=1)                                          # (128, 768)
    augb = np.stack([f["aug_br"], f["aug_bu"], f["aug_bh"]], axis=1)  # (128,3)
    outWT = np.ascontiguousarray(f["out_W"].reshape(5, D).T)          # (128,5)
    outb = f["out_b"].reshape(1, 1)

    shared_bf = dict(WihT=WihT, WhhT=WhhT, W0T=W0T, W1T=W1T, W2T=W2T,
                     augW=augW, outWT=outWT)
    shared = dict(bihT=bihT, bhhT=bhhT, b0=b0, b1=b1, b2=b2, augb=augb,
                  outb=outb)
    shared = {k: np.ascontiguousarray(v.astype(np.float32)) for k, v in
              shared.items()}
    shared.update({k: np.ascontiguousarray(v.astype(BF)) for k, v in
                   shared_bf.items()})
    blk = np.zeros((CG, CG * BL), dtype=BF)
    for s in range(CG):
        blk[s, s * BL:(s + 1) * BL] = 1
    shared["blkmask"] = blk

    in_maps = []
    for c in range(NCORES):
        s = slice(c * BL, (c + 1) * BL)
        m = dict(shared)
        m["hist"] = np.ascontiguousarray(
            f["item_historical_embedding"][s].astype(BF))
        m["item"] = np.ascontiguousarray(f["item_embedding"][s].astype(np.float32))
        m["user"] = np.ascontiguousarray(f["user_embedding"][s].astype(np.float32))
        m["maskd"] = np.ascontiguousarray(f["mask"][s].astype(np.float32))
        m["seqd"] = np.ascontiguousarray(
            f["sequential_length"][s].reshape(BL, 1).astype(np.int32))
        in_maps.append(m)
    return in_maps


def kernel(**inputs) -> np.ndarray:
    nc = _get_nc()
    in_maps = make_in_maps(inputs)
    res = run_bass_kernel_spmd(nc, in_maps, list(range(NCORES)))
    return np.concatenate(
        [np.asarray(res.results[c]["out"]).reshape(BL) for c in range(NCORES)])
